# revision 1
# baseline (speedup 1.0000x reference)
"""Trainium2 Bass kernel for the cosine-gated LSTM cell (CGLSTMCellv1).

Full inputs in, full outputs out. Internally: data-parallel shard of the
batch dim across 8 NeuronCores, weights replicated, no cross-core comms.

Math per core (rows = local batch slice):
  mapped = x @ Wm + bm
  attn   = sigmoid(cos_sim(mapped, hx));  s = 1 + attn
  gates  = concat(s*x, hx) @ W + b  = s*(x@Wx) + hx@Wh + b  (s folded into xT)
  i,f,g,o = LN-gates -> sigmoid/tanh
  cx_new = f*cx + i*g ; hx_new = o*tanh(cx_new)
  hx_mod = hx_new * (1 + sigmoid((cos_sim(hx_new,cx_new)+1)/2))

Walrus codegen limits sync waits per instruction (Matmult: 1, DMA: 2), so
the kernel is structured to keep semaphore fan-in low: PSUM tiles are
released by a single engine, x/hx are preloaded into dedicated tiles, the
gamma/beta replicas are consumed by gpsimd only, and dummy "absorber"
transposes pre-observe DMA semaphores before real matmuls need them.
"""

import numpy as np

B_FULL, DIM_I, DIM_H = 8192, 1024, 1024
NCORES = 8
BL = B_FULL // NCORES  # 1024 rows per core
P = 128
H4 = 4 * DIM_H
NKB1 = DIM_I // P           # 8  k-blocks for mm1
NKB2 = (DIM_I + DIM_H) // P  # 16 k-blocks for mm2
CHUNK = 256                 # W column chunk
NCH_G = DIM_H // CHUNK      # 4 chunks per gate
LN_EPS = 1e-5
COS_EPS2 = 1e-12

_cache = {}


def build_nc(nbt=BL // P):
    """Build the single-core Bass module; nbt = number of 128-row batch tiles."""
    from contextlib import ExitStack

    import concourse.bass as bass
    import concourse.mybir as mybir
    import concourse.tile as tile
    import concourse.tile_rust as tile_rust
    from concourse.masks import make_identity

    fp32 = mybir.dt.float32
    AF = mybir.ActivationFunctionType
    OP = mybir.AluOpType
    bl = nbt * P

    nc = bass.Bass()
    xd = nc.dram_tensor("x", [bl, DIM_I], fp32, kind="ExternalInput")
    hxd = nc.dram_tensor("hx", [bl, DIM_H], fp32, kind="ExternalInput")
    cxd = nc.dram_tensor("cx", [bl, DIM_H], fp32, kind="ExternalInput")
    Wd = nc.dram_tensor("W", [DIM_I + DIM_H, H4], fp32, kind="ExternalInput")
    bd = nc.dram_tensor("b", [H4], fp32, kind="ExternalInput")
    Wmd = nc.dram_tensor("Wm", [DIM_I, DIM_H], fp32, kind="ExternalInput")
    bmd = nc.dram_tensor("bm", [DIM_H], fp32, kind="ExternalInput")
    gd = nc.dram_tensor("gammas", [4, DIM_H], fp32, kind="ExternalInput")
    btd = nc.dram_tensor("betas", [4, DIM_H], fp32, kind="ExternalInput")
    hxo = nc.dram_tensor("hx_out", [bl, DIM_H], fp32, kind="ExternalOutput")
    cxo = nc.dram_tensor("cx_out", [bl, DIM_H], fp32, kind="ExternalOutput")

    def bcast_row(src_ap):
        # view an [N]-shaped AP as [P, N] with 0-step partition broadcast
        return bass.AP(
            tensor=src_ap.tensor, offset=src_ap.offset, ap=[[0, P]] + list(src_ap.ap)
        )

    def raw(inst):
        return getattr(inst, "ins", inst)

    with tile.TileContext(nc) as tc, ExitStack() as ctx:
        singles = ctx.enter_context(tc.tile_pool(name="singles", bufs=1))

        ident = singles.tile([P, P], fp32)
        make_identity(nc, ident)
        ones128 = singles.tile([P, P], fp32)
        nc.vector.memset(ones128, 1.0)
        zrow = singles.tile([P, P], fp32)
        nc.vector.memset(zrow, 0.0)
        epsln = singles.tile([P, 1], fp32)
        nc.vector.memset(epsln, LN_EPS)
        halfc = singles.tile([P, 1], fp32)
        i_msl = nc.vector.memset(halfc, 0.5)
        bm_rep = singles.tile([P, DIM_H], fp32)
        i_bm = nc.sync.dma_start(out=bm_rep, in_=bcast_row(bmd[:]))

        # transposed activations, persistent across both phases
        xsT_all = singles.tile([P, nbt, NKB1, P], fp32)
        hxT_all = singles.tile([P, nbt, NKB1, P], fp32)

        Wv = Wd[:].rearrange("(kb p) n -> p kb n", p=P)
        Wmv = Wmd[:].rearrange("(kb p) n -> p kb n", p=P)

        def absorber(ps_tile):
            def absorb(dep_inst=None):
                """Dummy PE transpose pre-observing one semaphore so real
                matmuls never need more than one sync wait (S3_LW limit)."""
                di = nc.tensor.transpose(ps_tile, ident, ident)
                if dep_inst is not None:
                    tile_rust.add_dep_helper(
                        raw(di), raw(dep_inst), reason="absorb sem for PE"
                    )
                return di

            return absorb

        # ---------------- phase 1 ----------------
        with ExitStack() as p1:
            wm_pool = p1.enter_context(tc.tile_pool(name="wm", bufs=1))
            io_pool = p1.enter_context(tc.tile_pool(name="io1", bufs=2))
            sm_pool = p1.enter_context(tc.tile_pool(name="smalls1", bufs=4))
            dump_pool = p1.enter_context(tc.tile_pool(name="dump1", bufs=2))
            ps_aux = p1.enter_context(tc.tile_pool(name="psaux", bufs=3, space="PSUM"))
            ps_m1 = p1.enter_context(tc.tile_pool(name="psm1", bufs=2, space="PSUM"))

            wm_sb = wm_pool.tile([P, NKB1, DIM_H], fp32)
            i_wm = nc.sync.dma_start(out=wm_sb, in_=Wmv)

            x_all = wm_pool.tile([P, nbt, DIM_I], fp32)
            hx_all = wm_pool.tile([P, nbt, DIM_H], fp32)
            xload, hxload = [], []
            for t in range(nbt):
                xload.append(
                    nc.sync.dma_start(
                        out=x_all[:, t], in_=xd[t * P : (t + 1) * P, :]
                    )
                )
                hxload.append(
                    nc.sync.dma_start(
                        out=hx_all[:, t], in_=hxd[t * P : (t + 1) * P, :]
                    )
                )

            dmy = ps_aux.tile([P, P], fp32, tag="dmy", bufs=1, name="dmy")
            absorb = absorber(dmy)
            absorb()  # ident (gpsimd sem)
            absorb(i_msl)  # vector memsets
            absorb(i_bm)  # bm_rep dma queue
            absorb(i_wm)  # wm dma queue

            cp_insts = []
            for t in range(nbt):
                x_t = x_all[:, t]
                hx_t = hx_all[:, t]

                absorb(xload[t])
                xT_t = io_pool.tile([P, NKB1, P], fp32, tag="xT_t")
                for j in range(NKB1):
                    pt = ps_aux.tile([P, P], fp32, tag="paux", name=f"ptx{t}_{j}")
                    nc.tensor.transpose(pt, x_t[:, j * P : (j + 1) * P], ident)
                    nc.scalar.copy(xT_t[:, j, :], pt)
                absorb(hxload[t])
                for j in range(NKB1):
                    pt = ps_aux.tile([P, P], fp32, tag="paux", name=f"pth{t}_{j}")
                    nc.tensor.transpose(pt, hx_t[:, j * P : (j + 1) * P], ident)
                    nc.scalar.copy(hxT_all[:, t, j, :], pt)

                # mm1: mapped = bm + x @ Wm   (psum [P, 1024], two N=512 groups)
                if t >= 2:
                    absorb(cp_insts[t - 2])  # pm slot release (DVE copy)
                pm = ps_m1.tile([P, DIM_H], fp32, tag="pm1", name=f"pm{t}")
                for nh in range(2):
                    cs = slice(nh * 512, (nh + 1) * 512)
                    nc.tensor.matmul(
                        pm[:, cs], ident, bm_rep[:, cs], start=True, stop=False
                    )
                    for kb in range(NKB1):
                        nc.tensor.matmul(
                            pm[:, cs],
                            xT_t[:, kb, :],
                            wm_sb[:, kb, cs],
                            start=False,
                            stop=(kb == NKB1 - 1),
                        )

                # cosine attention gate; single DVE reader releases the psum
                map_sb = io_pool.tile([P, DIM_H], fp32, tag="map_sb")
                cp_insts.append(nc.vector.tensor_copy(map_sb, pm))
                dot_t = sm_pool.tile([P, 1], fp32, tag="dot")
                dmp0 = dump_pool.tile([P, DIM_H], fp32, tag="dump")
                nc.vector.scalar_tensor_tensor(
                    out=dmp0,
                    in0=map_sb,
                    scalar=1.0,
                    in1=hx_t,
                    op0=OP.mult,
                    op1=OP.mult,
                    accum_out=dot_t,
                )
                sqm_t = sm_pool.tile([P, 1], fp32, tag="sqm")
                dmp1 = dump_pool.tile([P, DIM_H], fp32, tag="dump")
                nc.scalar.activation(dmp1, map_sb, AF.Square, accum_out=sqm_t)
                sqh_t = sm_pool.tile([P, 1], fp32, tag="sqh")
                dmp2 = dump_pool.tile([P, DIM_H], fp32, tag="dump")
                nc.scalar.activation(dmp2, hx_t, AF.Square, accum_out=sqh_t)

                m1_t = sm_pool.tile([P, 1], fp32, tag="m1")
                nc.vector.tensor_scalar_max(m1_t, sqm_t, COS_EPS2)
                m2_t = sm_pool.tile([P, 1], fp32, tag="m2")
                nc.vector.tensor_scalar_max(m2_t, sqh_t, COS_EPS2)
                den_t = sm_pool.tile([P, 1], fp32, tag="den")
                nc.vector.tensor_tensor(den_t, m1_t, m2_t, OP.mult)
                sd_t = sm_pool.tile([P, 1], fp32, tag="sd")
                nc.scalar.sqrt(sd_t, den_t)
                rinv_t = sm_pool.tile([P, 1], fp32, tag="rinv")
                nc.vector.reciprocal(rinv_t, sd_t)
                cos_t = sm_pool.tile([P, 1], fp32, tag="cos")
                nc.vector.tensor_scalar_mul(cos_t, dot_t, rinv_t)
                attn_t = sm_pool.tile([P, 1], fp32, tag="attn")
                nc.scalar.activation(attn_t, cos_t, AF.Sigmoid)
                s1_t = sm_pool.tile([P, 1], fp32, tag="s1")
                nc.scalar.add(s1_t, attn_t, 1.0)

                # transpose s -> row 0 of zrow, replicate via ones-matmul
                psT = ps_aux.tile([1, P], fp32, tag="paux", name=f"psT{t}")
                nc.tensor.transpose(psT, s1_t, ident)
                nc.scalar.copy(zrow[0:1, :], psT)
                psr = ps_aux.tile([P, P], fp32, tag="paux", name=f"psr{t}")
                nc.tensor.matmul(psr, ones128, zrow, start=True, stop=True)
                srep_t = sm_pool.tile([P, P], fp32, tag="srep")
                nc.scalar.copy(srep_t, psr)

                srep_brd = bass.AP(
                    tensor=srep_t.tensor,
                    offset=srep_t.offset,
                    ap=[list(srep_t.ap[0]), [0, NKB1], list(srep_t.ap[1])],
                )
                nc.vector.tensor_tensor(xsT_all[:, t], xT_t, srep_brd, OP.mult)

        tc.strict_bb_all_engine_barrier()

        # ---------------- phase 2 ----------------
        with ExitStack() as p2:
            w_pool = p2.enter_context(tc.tile_pool(name="wch", bufs=2))
            bsl_pool = p2.enter_context(tc.tile_pool(name="bsl", bufs=2))
            gb_pool = p2.enter_context(tc.tile_pool(name="gb", bufs=1))
            iact_pool = p2.enter_context(tc.tile_pool(name="iact", bufs=nbt))
            zst_pool = p2.enter_context(tc.tile_pool(name="zst", bufs=nbt + 1))
            z3_pool = p2.enter_context(tc.tile_pool(name="z3", bufs=3))
            st_pool = p2.enter_context(tc.tile_pool(name="stats", bufs=nbt + 2))
            ga_pool = p2.enter_context(tc.tile_pool(name="gact", bufs=2))
            cx_pool = p2.enter_context(tc.tile_pool(name="cxin", bufs=2))
            sm2_pool = p2.enter_context(tc.tile_pool(name="smalls2", bufs=2))
            ps_g = p2.enter_context(tc.tile_pool(name="psg", bufs=3, space="PSUM"))
            ps_dump = p2.enter_context(
                tc.tile_pool(name="psdump", bufs=2, space="PSUM")
            )

            dmy2 = ps_g.tile([P, P], fp32, tag="dmy2", bufs=1, name="dmy2")
            absorb2 = absorber(dmy2)

            iact = [
                iact_pool.tile([P, DIM_H], fp32, tag="iact", name=f"iact{t}")
                for t in range(nbt)
            ]
            zst = [
                zst_pool.tile([P, 3 * CHUNK], fp32, tag="zst", name=f"zst{t}")
                for t in range(nbt)
            ]
            stats = [
                st_pool.tile([P, NCH_G, 6], fp32, tag="st", name=f"st{t}")
                for t in range(nbt)
            ]

            # gate order: i first (stored), then g (i*g), f (cx_new), o (outputs)
            for gi, func, role in (
                (0, AF.Sigmoid, "i"),
                (2, AF.Tanh, "g"),
                (1, AF.Sigmoid, "f"),
                (3, AF.Sigmoid, "o"),
            ):
                grep_t = gb_pool.tile([P, DIM_H], fp32, tag="grep", name=f"grep{gi}")
                nc.sync.dma_start(out=grep_t, in_=bcast_row(gd[gi, :]))
                brep_t = gb_pool.tile([P, DIM_H], fp32, tag="brep", name=f"brep{gi}")
                nc.sync.dma_start(out=brep_t, in_=bcast_row(btd[gi, :]))

                for c in range(NCH_G):
                    col0 = gi * DIM_H + c * CHUNK
                    wch = w_pool.tile(
                        [P, NKB2, CHUNK], fp32, tag="wch", name=f"wch{gi}_{c}"
                    )
                    nc.sync.dma_start(out=wch, in_=Wv[:, :, col0 : col0 + CHUNK])
                    bsl = bsl_pool.tile(
                        [P, CHUNK], fp32, tag="bsl", name=f"bsl{gi}_{c}"
                    )
                    i_bsl = nc.sync.dma_start(
                        out=bsl, in_=bcast_row(bd[col0 : col0 + CHUNK])
                    )
                    absorb2(i_bsl)

                    for t in range(nbt):
                        ps = ps_g.tile(
                            [P, CHUNK], fp32, tag="pg", name=f"pg{gi}_{c}_{t}"
                        )
                        nc.tensor.matmul(ps, ident, bsl, start=True, stop=False)
                        for kb in range(NKB2):
                            lhsT = (
                                xsT_all[:, t, kb, :]
                                if kb < NKB1
                                else hxT_all[:, t, kb - NKB1, :]
                            )
                            nc.tensor.matmul(
                                ps,
                                lhsT,
                                wch[:, kb, :],
                                start=False,
                                stop=(kb == NKB2 - 1),
                            )
                        if c < NCH_G - 1:
                            zpart = zst[t][:, c * CHUNK : (c + 1) * CHUNK]
                            nc.scalar.copy(zpart, ps)
                            nc.vector.bn_stats(stats[t][:, c, :], zpart)
                        else:
                            z3_t = z3_pool.tile(
                                [P, CHUNK], fp32, tag="z3", name=f"z3_{gi}_{t}"
                            )
                            nc.scalar.copy(z3_t, ps)
                            nc.vector.bn_stats(stats[t][:, c, :], z3_t)

                            mv_t = sm2_pool.tile([P, 2], fp32, tag="mv")
                            nc.vector.bn_aggr(mv_t, stats[t])
                            sd2_t = sm2_pool.tile([P, 1], fp32, tag="sd2")
                            nc.scalar.activation(
                                sd2_t, mv_t[:, 1:2], AF.Sqrt, bias=epsln
                            )
                            rstd_t = sm2_pool.tile([P, 1], fp32, tag="rstd")
                            nc.vector.reciprocal(rstd_t, sd2_t)
                            nmu_t = sm2_pool.tile([P, 1], fp32, tag="nmu")
                            nc.vector.tensor_scalar(
                                nmu_t, mv_t[:, 0:1], rstd_t, -1.0, OP.mult, OP.mult
                            )
                            if role == "i":
                                ga = iact[t]
                            else:
                                ga = ga_pool.tile(
                                    [P, DIM_H], fp32, tag="ga", name=f"ga{gi}_{t}"
                                )
                            nc.vector.tensor_scalar(
                                ga[:, 0 : 3 * CHUNK],
                                zst[t],
                                rstd_t,
                                nmu_t,
                                OP.mult,
                                OP.add,
                            )
                            nc.vector.tensor_scalar(
                                ga[:, 3 * CHUNK : DIM_H],
                                z3_t,
                                rstd_t,
                                nmu_t,
                                OP.mult,
                                OP.add,
                            )
                            nc.gpsimd.tensor_tensor(ga, ga, grep_t, OP.mult)
                            nc.gpsimd.tensor_tensor(ga, ga, brep_t, OP.add)
                            nc.scalar.activation(ga, ga, func)

                            if role == "g":
                                nc.vector.tensor_tensor(iact[t], iact[t], ga, OP.mult)
                            elif role == "f":
                                cx_t = cx_pool.tile(
                                    [P, DIM_H], fp32, tag="cx", name=f"cx{t}"
                                )
                                nc.sync.dma_start(
                                    out=cx_t, in_=cxd[t * P : (t + 1) * P, :]
                                )
                                nc.gpsimd.tensor_tensor(cx_t, ga, cx_t, OP.mult)
                                nc.gpsimd.tensor_tensor(iact[t], iact[t], cx_t, OP.add)
                                nc.scalar.dma_start(
                                    out=cxo[t * P : (t + 1) * P, :], in_=iact[t]
                                )
                            elif role == "o":
                                tnh_t = ga_pool.tile(
                                    [P, DIM_H], fp32, tag="tnh", name=f"tnh{t}"
                                )
                                nc.scalar.activation(tnh_t, iact[t], AF.Tanh)
                                # hx_new in place of tanh(cx_new)
                                hxn_t = tnh_t
                                nc.gpsimd.tensor_tensor(hxn_t, ga, tnh_t, OP.mult)

                                # second cosine gate
                                dot2 = sm2_pool.tile([P, 1], fp32, tag="dot2")
                                dmp = ps_dump.tile(
                                    [P, DIM_H], fp32, tag="dmp", name=f"dmp{t}"
                                )
                                nc.vector.scalar_tensor_tensor(
                                    out=dmp,
                                    in0=hxn_t,
                                    scalar=1.0,
                                    in1=iact[t],
                                    op0=OP.mult,
                                    op1=OP.mult,
                                    accum_out=dot2,
                                )
                                sq1 = sm2_pool.tile([P, 1], fp32, tag="sq1")
                                dmpa = ps_dump.tile(
                                    [P, DIM_H], fp32, tag="dmp", name=f"dmpa{t}"
                                )
                                nc.scalar.activation(
                                    dmpa, hxn_t, AF.Square, accum_out=sq1
                                )
                                sq2 = sm2_pool.tile([P, 1], fp32, tag="sq2")
                                dmpb = ps_dump.tile(
                                    [P, DIM_H], fp32, tag="dmp", name=f"dmpb{t}"
                                )
                                nc.scalar.activation(
                                    dmpb, iact[t], AF.Square, accum_out=sq2
                                )
                                ma = sm2_pool.tile([P, 1], fp32, tag="ma")
                                nc.vector.tensor_scalar_max(ma, sq1, COS_EPS2)
                                mb = sm2_pool.tile([P, 1], fp32, tag="mb")
                                nc.vector.tensor_scalar_max(mb, sq2, COS_EPS2)
                                dn2 = sm2_pool.tile([P, 1], fp32, tag="dn2")
                                nc.vector.tensor_tensor(dn2, ma, mb, OP.mult)
                                sdd = sm2_pool.tile([P, 1], fp32, tag="sdd")
                                nc.scalar.sqrt(sdd, dn2)
                                rr2 = sm2_pool.tile([P, 1], fp32, tag="rr2")
                                nc.vector.reciprocal(rr2, sdd)
                                arg2 = sm2_pool.tile([P, 1], fp32, tag="arg2")
                                nc.vector.tensor_scalar(
                                    arg2, dot2, rr2, 0.5, OP.mult, OP.mult
                                )
                                co_t = sm2_pool.tile([P, 1], fp32, tag="co")
                                nc.scalar.activation(
                                    co_t, arg2, AF.Sigmoid, bias=halfc
                                )
                                nc.vector.tensor_scalar_add(co_t, co_t, 1.0)
                                nc.vector.tensor_scalar_mul(hxn_t, hxn_t, co_t)
                                nc.scalar.dma_start(
                                    out=hxo[t * P : (t + 1) * P, :], in_=hxn_t
                                )
    _split_excess_waits(nc)
    return nc


def _split_excess_waits(nc):
    """Walrus ISA structs have limited sync-wait slots (Matmult/LDW: 1,
    DMA: 2, several DVE/ACT structs: 1-2). The Tile scheduler can emit more.
    Move excess waits onto standalone EventSemaphore instructions injected
    just before the offender on the same engine."""
    import concourse.mybir as mybir

    caps = {}
    skip = {"EventSemaphore", "RegisterMove", "UnconditionalBranch"}
    n_split = 0
    for fn in nc.m.functions:
        for blk in fn.blocks:
            out = []
            changed = False
            for ins in blk.instructions:
                si = ins.sync_info
                op = ins.concise_opcode() if callable(
                    getattr(ins, "concise_opcode", None)
                ) else None
                opname = type(ins).__name__.replace("Inst", "", 1)
                if (
                    si is not None
                    and si.on_wait
                    and opname not in skip
                    and len(si.on_wait) > caps.get(opname, 1)
                ):
                    cap = caps.get(opname, 1)
                    waits = list(si.on_wait)
                    excess, keep = waits[:-cap], waits[-cap:]
                    for k, w in enumerate(excess):
                        ev = mybir.InstEventSemaphore(
                            name=f"{ins.name}-wsp{k}",
                            ins=[],
                            outs=[],
                            sync_info=mybir.SyncInfo(on_wait=[w], on_update=[]),
                        )
                        ev.engine = ins.engine
                        out.append(ev)
                        n_split += 1
                    ins.sync_info = mybir.SyncInfo(
                        on_wait=keep, on_update=list(si.on_update)
                    )
                    changed = True
                out.append(ins)
            if changed:
                blk.instructions = out
    return n_split


def _get_nc():
    if "nc" not in _cache:
        _cache["nc"] = build_nc()
    return _cache["nc"]


def kernel(x, hx, cx, W, b, Wm, bm, gammas, betas):
    from concourse.bass_utils import run_bass_kernel_spmd

    nc = _get_nc()
    x = np.ascontiguousarray(np.asarray(x, np.float32))
    hx = np.ascontiguousarray(np.asarray(hx, np.float32))
    cx = np.ascontiguousarray(np.asarray(cx, np.float32))
    shared = {
        "W": np.ascontiguousarray(np.asarray(W, np.float32)),
        "b": np.ascontiguousarray(np.asarray(b, np.float32)),
        "Wm": np.ascontiguousarray(np.asarray(Wm, np.float32)),
        "bm": np.ascontiguousarray(np.asarray(bm, np.float32)),
        "gammas": np.ascontiguousarray(np.asarray(gammas, np.float32)),
        "betas": np.ascontiguousarray(np.asarray(betas, np.float32)),
    }
    in_maps = []
    for i in range(NCORES):
        sl = slice(i * BL, (i + 1) * BL)
        in_maps.append({"x": x[sl], "hx": hx[sl], "cx": cx[sl], **shared})
    res = run_bass_kernel_spmd(nc, in_maps, list(range(NCORES)))
    hx_mod = np.concatenate([r["hx_out"] for r in res.results], axis=0)
    cx_new = np.concatenate([r["cx_out"] for r in res.results], axis=0)
    return (hx_mod, cx_new)



# revision 7
# speedup vs baseline: 2.4064x; 2.4064x over previous
"""Trainium2 Bass kernel for the cosine-gated LSTM cell (CGLSTMCellv1).

Full inputs in, full outputs out. Internally: data-parallel shard of the
batch dim across 8 NeuronCores, weights replicated, no cross-core comms.

Math per core (rows = local batch slice):
  mapped = x @ Wm + bm
  attn   = sigmoid(cos_sim(mapped, hx));  s = 1 + attn
  gates  = concat(s*x, hx) @ W + b  = s*(x@Wx) + hx@Wh + b  (s folded into xT)
  i,f,g,o = LN-gates -> sigmoid/tanh
  cx_new = f*cx + i*g ; hx_new = o*tanh(cx_new)
  hx_mod = hx_new * (1 + sigmoid((cos_sim(hx_new,cx_new)+1)/2))

Perf notes vs the fp32 baseline:
  - All GEMM matmuls run as float32r (1 cycle/row when N>=256, vs 4 for
    fp32) via AP.bitcast; PSUM accumulation stays fp32.
  - No ACT sqrt anywhere: rsqrt is a Quake-style bit hack + Newton steps
    on the Vector engine, so the ScalarE activation table stays on the
    sigmoid_and_others set (sigmoid/tanh/square/copy) the whole kernel —
    the baseline burned ~100us in ACT_TABLE_LOADs alternating sqrt<->
    sigmoid.
  - LayerNorm tail consumes PSUM directly: bn_stats on the psum chunks,
    final chunk never copied; apply is two fused scalar_tensor_tensor
    passes  u=(z-mu)*gamma ; w=u*rstd+beta  on DVE, activation on ACT.
  - GpSimd (no PSUM port) only does SBUF-side elementwise (gate combine).

Walrus codegen limits sync waits per instruction (Matmult: 1, DMA: 2), so
PSUM consumers are kept few, and dummy "absorber" transposes pre-observe
DMA semaphores; _split_excess_waits catches the rest.
"""

import numpy as np

B_FULL, DIM_I, DIM_H = 8192, 1024, 1024
NCORES = 8
BL = B_FULL // NCORES  # 1024 rows per core
P = 128
H4 = 4 * DIM_H
NKB1 = DIM_I // P            # 8  k-blocks for mm1
NKB2 = (DIM_I + DIM_H) // P  # 16 k-blocks for mm2
CHUNK = 256                  # W column chunk
NCH_G = DIM_H // CHUNK       # 4 chunks per gate
LN_EPS = 1e-5
COS_EPS2 = 1e-12
QMAGIC = 0x5F3759DF

_cache = {}


def build_nc(nbt=BL // P, split_waits=True):
    """Build the single-core Bass module; nbt = number of 128-row batch tiles."""
    from contextlib import ExitStack

    import concourse.bass as bass
    import concourse.mybir as mybir
    import concourse.tile as tile
    import concourse.tile_rust as tile_rust
    from concourse.masks import make_identity

    fp32 = mybir.dt.float32
    fp32r = mybir.dt.float32r
    i32 = mybir.dt.int32
    AF = mybir.ActivationFunctionType
    OP = mybir.AluOpType
    bl = nbt * P

    def R(ap):
        return ap.bitcast(fp32r)

    nc = bass.Bass()
    xd = nc.dram_tensor("x", [bl, DIM_I], fp32, kind="ExternalInput")
    hxd = nc.dram_tensor("hx", [bl, DIM_H], fp32, kind="ExternalInput")
    cxd = nc.dram_tensor("cx", [bl, DIM_H], fp32, kind="ExternalInput")
    Wd = nc.dram_tensor("W", [DIM_I + DIM_H, H4], fp32r, kind="ExternalInput")
    bd = nc.dram_tensor("b", [H4], fp32r, kind="ExternalInput")
    Wmd = nc.dram_tensor("Wm", [DIM_I, DIM_H], fp32r, kind="ExternalInput")
    bmd = nc.dram_tensor("bm", [DIM_H], fp32r, kind="ExternalInput")
    gd = nc.dram_tensor("gammas", [4, DIM_H], fp32, kind="ExternalInput")
    btd = nc.dram_tensor("betas", [4, DIM_H], fp32, kind="ExternalInput")
    hxo = nc.dram_tensor("hx_out", [bl, DIM_H], fp32, kind="ExternalOutput")
    cxo = nc.dram_tensor("cx_out", [bl, DIM_H], fp32, kind="ExternalOutput")

    def bcast_row(src_ap):
        # view an [N]-shaped AP as [P, N] with 0-step partition broadcast
        return bass.AP(
            tensor=src_ap.tensor, offset=src_ap.offset, ap=[[0, P]] + list(src_ap.ap)
        )

    def raw(inst):
        return getattr(inst, "ins", inst)

    with tile.TileContext(nc) as tc, ExitStack() as ctx:
        singles = ctx.enter_context(tc.tile_pool(name="singles", bufs=1))

        ident = singles.tile([P, P], fp32)
        make_identity(nc, ident)
        ident_r = singles.tile([P, P], fp32r)
        nc.scalar.copy(ident_r, ident)
        ones128 = singles.tile([P, P], fp32)
        nc.vector.memset(ones128, 1.0)
        zrow = singles.tile([P, P], fp32)
        nc.vector.memset(zrow, 0.0)
        halfc = singles.tile([P, 1], fp32)
        nc.vector.memset(halfc, 0.5)
        one_i = singles.tile([P, 1], i32)
        nc.vector.memset(one_i, 1)
        magic_i = singles.tile([P, 1], i32)
        i_msl = nc.vector.memset(magic_i, QMAGIC)

        # transposed activations, persistent across both phases
        xsT_all = singles.tile([P, nbt, NKB1, P], fp32r)
        hxT_all = singles.tile([P, nbt, NKB1, P], fp32r)

        Wv = Wd[:].rearrange("(kb p) n -> p kb n", p=P)
        Wmv = Wmd[:].rearrange("(kb p) n -> p kb n", p=P)

        def rsqrt_dve(pool, v_ap, iters, tag):
            """1/sqrt(v) on DVE: Quake bit hack + `iters` Newton steps.
            v_ap: [P,1] fp32 AP. Returns a [P,1] fp32 tile."""
            vi = v_ap.bitcast(i32)
            y = pool.tile([P, 1], fp32, tag=f"{tag}_y")
            yi = y.bitcast(i32)
            t0 = pool.tile([P, 1], i32, tag=f"{tag}_t0")
            nc.vector.tensor_tensor(t0, vi, one_i, OP.logical_shift_right)
            nc.vector.tensor_tensor(yi, magic_i, t0, OP.subtract)
            for _ in range(iters):
                a = pool.tile([P, 1], fp32, tag=f"{tag}_a")
                nc.vector.tensor_tensor(a, v_ap, y, OP.mult)
                nc.vector.tensor_tensor(a, a, y, OP.mult)
                nc.vector.tensor_scalar(a, a, -0.5, 1.5, OP.mult, OP.add)
                nc.vector.tensor_tensor(y, y, a, OP.mult)
            return y

        def absorber(ps_tile):
            def absorb(dep_inst=None):
                """Dummy PE transpose pre-observing one semaphore so real
                matmuls never need more than one sync wait (S3_LW limit)."""
                di = nc.tensor.transpose(ps_tile, ident, ident)
                if dep_inst is not None:
                    tile_rust.add_dep_helper(
                        raw(di), raw(dep_inst), reason="absorb sem for PE"
                    )
                return di

            return absorb

        # ---------------- phase 1 ----------------
        with ExitStack() as p1:
            wm_pool = p1.enter_context(tc.tile_pool(name="wm", bufs=1))
            io_pool = p1.enter_context(tc.tile_pool(name="io1", bufs=2))
            sm_pool = p1.enter_context(tc.tile_pool(name="smalls1", bufs=4))
            dump_pool = p1.enter_context(tc.tile_pool(name="dump1", bufs=3))
            ps_tr = p1.enter_context(tc.tile_pool(name="pstr", bufs=2, space="PSUM"))
            ps_sm = p1.enter_context(tc.tile_pool(name="pssm", bufs=1, space="PSUM"))
            ps_m1 = p1.enter_context(tc.tile_pool(name="psm1", bufs=2, space="PSUM"))

            bm_rep = wm_pool.tile([P, DIM_H], fp32r)
            i_bm = nc.sync.dma_start(out=bm_rep, in_=bcast_row(bmd[:]))
            wm_sb = wm_pool.tile([P, NKB1, DIM_H], fp32r)
            i_wm = nc.sync.dma_start(out=wm_sb, in_=Wmv)

            x_all = wm_pool.tile([P, nbt, DIM_I], fp32)
            hx_all = wm_pool.tile([P, nbt, DIM_H], fp32)
            xload, hxload = [], []
            for t in range(nbt):
                xload.append(
                    nc.sync.dma_start(
                        out=x_all[:, t], in_=xd[t * P : (t + 1) * P, :]
                    )
                )
                hxload.append(
                    nc.sync.dma_start(
                        out=hx_all[:, t], in_=hxd[t * P : (t + 1) * P, :]
                    )
                )

            dmy = ps_sm.tile([P, P], fp32, tag="dmy", bufs=1, name="dmy")
            absorb = absorber(dmy)
            absorb()  # ident (gpsimd sem)
            absorb(i_msl)  # vector memsets
            absorb(i_bm)  # bm_rep dma queue
            absorb(i_wm)  # wm dma queue

            cp_insts = []
            for t in range(nbt):
                x_t = x_all[:, t]
                hx_t = hx_all[:, t]

                absorb(xload[t])
                xT_t = io_pool.tile([P, NKB1, P], fp32r, tag="xT_t")
                for h in range(2):
                    pt = ps_tr.tile([P, 512], fp32, tag="tr", name=f"ptx{t}_{h}")
                    for j in range(4):
                        jj = h * 4 + j
                        nc.tensor.transpose(
                            pt[:, j * P : (j + 1) * P],
                            x_t[:, jj * P : (jj + 1) * P],
                            ident,
                        )
                    nc.scalar.copy(xT_t[:, h * 4 : (h + 1) * 4, :], pt)
                absorb(hxload[t])
                for h in range(2):
                    pt = ps_tr.tile([P, 512], fp32, tag="tr", name=f"pth{t}_{h}")
                    for j in range(4):
                        jj = h * 4 + j
                        nc.tensor.transpose(
                            pt[:, j * P : (j + 1) * P],
                            hx_t[:, jj * P : (jj + 1) * P],
                            ident,
                        )
                    nc.scalar.copy(hxT_all[:, t, h * 4 : (h + 1) * 4, :], pt)

                # mm1: mapped = bm + x @ Wm   (psum [P, 1024], two N=512 groups)
                if t >= 2:
                    for ci in cp_insts[t - 2]:
                        absorb(ci)  # pm slot release (DVE dot + ACT square)
                pm = ps_m1.tile([P, DIM_H], fp32, tag="pm1", name=f"pm{t}")
                for nh in range(2):
                    cs = slice(nh * 512, (nh + 1) * 512)
                    nc.tensor.matmul(
                        pm[:, cs], ident_r, bm_rep[:, cs], start=True, stop=False
                    )
                    for kb in range(NKB1):
                        nc.tensor.matmul(
                            pm[:, cs],
                            xT_t[:, kb, :],
                            wm_sb[:, kb, cs],
                            start=False,
                            stop=(kb == NKB1 - 1),
                        )

                # cosine attention gate; DVE dot + ACT square read the psum
                dot_t = sm_pool.tile([P, 1], fp32, tag="dot")
                dmp0 = dump_pool.tile([P, DIM_H], fp32, tag="dump")
                i_dot = nc.vector.scalar_tensor_tensor(
                    out=dmp0,
                    in0=pm,
                    scalar=1.0,
                    in1=hx_t,
                    op0=OP.mult,
                    op1=OP.mult,
                    accum_out=dot_t,
                )
                sqm_t = sm_pool.tile([P, 1], fp32, tag="sqm")
                dmp1 = dump_pool.tile([P, DIM_H], fp32, tag="dump")
                i_sqm = nc.scalar.activation(dmp1, pm, AF.Square, accum_out=sqm_t)
                cp_insts.append((i_dot, i_sqm))
                sqh_t = sm_pool.tile([P, 1], fp32, tag="sqh")
                dmp2 = dump_pool.tile([P, DIM_H], fp32, tag="dump")
                nc.scalar.activation(dmp2, hx_t, AF.Square, accum_out=sqh_t)

                m1_t = sm_pool.tile([P, 1], fp32, tag="m1")
                nc.vector.tensor_scalar_max(m1_t, sqm_t, COS_EPS2)
                m2_t = sm_pool.tile([P, 1], fp32, tag="m2")
                nc.vector.tensor_scalar_max(m2_t, sqh_t, COS_EPS2)
                den_t = sm_pool.tile([P, 1], fp32, tag="den")
                nc.vector.tensor_tensor(den_t, m1_t, m2_t, OP.mult)
                rinv_t = rsqrt_dve(sm_pool, den_t, 2, "rs1")
                cos_t = sm_pool.tile([P, 1], fp32, tag="cos")
                nc.vector.tensor_scalar_mul(cos_t, dot_t, rinv_t)
                attn_t = sm_pool.tile([P, 1], fp32, tag="attn")
                nc.scalar.activation(attn_t, cos_t, AF.Sigmoid)
                s1_t = sm_pool.tile([P, 1], fp32, tag="s1")
                nc.scalar.add(s1_t, attn_t, 1.0)

                # transpose s -> row 0 of zrow, replicate via ones-matmul
                psT = ps_sm.tile([1, P], fp32, tag="paux", name=f"psT{t}")
                nc.tensor.transpose(psT, s1_t, ident)
                nc.scalar.copy(zrow[0:1, :], psT)
                psr = ps_sm.tile([P, P], fp32, tag="paux", name=f"psr{t}")
                nc.tensor.matmul(psr, ones128, zrow, start=True, stop=True)
                srep_t = sm_pool.tile([P, P], fp32, tag="srep")
                nc.scalar.copy(srep_t, psr)

                srep_brd = bass.AP(
                    tensor=srep_t.tensor,
                    offset=srep_t.offset,
                    ap=[list(srep_t.ap[0]), [0, NKB1], list(srep_t.ap[1])],
                )
                nc.vector.tensor_tensor(xsT_all[:, t], xT_t, srep_brd, OP.mult)

        tc.strict_bb_all_engine_barrier()

        # ---------------- phase 2 ----------------
        with ExitStack() as p2:
            w_pool = p2.enter_context(tc.tile_pool(name="wch", bufs=2))
            bsl_pool = p2.enter_context(tc.tile_pool(name="bsl", bufs=2))
            gb_pool = p2.enter_context(tc.tile_pool(name="gb", bufs=2))
            iact_pool = p2.enter_context(tc.tile_pool(name="iact", bufs=nbt))
            zst_pool = p2.enter_context(tc.tile_pool(name="zst", bufs=nbt))
            u_pool = p2.enter_context(tc.tile_pool(name="u", bufs=3))
            tnh_pool = p2.enter_context(tc.tile_pool(name="tnh", bufs=1))
            st_pool = p2.enter_context(tc.tile_pool(name="stats", bufs=nbt + 2))
            cx_pool = p2.enter_context(tc.tile_pool(name="cxin", bufs=2))
            dv_pool = p2.enter_context(tc.tile_pool(name="dvdump", bufs=2))
            sm2_pool = p2.enter_context(tc.tile_pool(name="smalls2", bufs=2))
            ps_g = p2.enter_context(tc.tile_pool(name="psg", bufs=4, space="PSUM"))
            ps_ad = p2.enter_context(
                tc.tile_pool(name="psact", bufs=1, space="PSUM")
            )

            dmy2 = ps_g.tile([P, P], fp32, tag="dmy2", bufs=1, name="dmy2")
            absorb2 = absorber(dmy2)

            iact = [
                iact_pool.tile([P, DIM_H], fp32, tag="iact", name=f"iact{t}")
                for t in range(nbt)
            ]
            zst = [
                zst_pool.tile([P, 3 * CHUNK], fp32, tag="zst", name=f"zst{t}")
                for t in range(nbt)
            ]
            stats = [
                st_pool.tile([P, NCH_G, 6], fp32, tag="st", name=f"st{t}")
                for t in range(nbt)
            ]

            # gate order: i first (stored), then g (i*g), f (cx_new), o (outputs)
            for gi, func, role in (
                (0, AF.Sigmoid, "i"),
                (2, AF.Tanh, "g"),
                (1, AF.Sigmoid, "f"),
                (3, AF.Sigmoid, "o"),
            ):
                grep_t = gb_pool.tile([P, DIM_H], fp32, tag="grep", name=f"grep{gi}")
                nc.sync.dma_start(out=grep_t, in_=bcast_row(gd[gi, :]))
                brep_t = gb_pool.tile([P, DIM_H], fp32, tag="brep", name=f"brep{gi}")
                nc.sync.dma_start(out=brep_t, in_=bcast_row(btd[gi, :]))

                for c in range(NCH_G):
                    col0 = gi * DIM_H + c * CHUNK
                    wch = w_pool.tile(
                        [P, NKB2, CHUNK], fp32r, tag="wch", name=f"wch{gi}_{c}"
                    )
                    nc.sync.dma_start(out=wch, in_=Wv[:, :, col0 : col0 + CHUNK])
                    bsl = bsl_pool.tile(
                        [P, CHUNK], fp32r, tag="bsl", name=f"bsl{gi}_{c}"
                    )
                    i_bsl = nc.sync.dma_start(
                        out=bsl, in_=bcast_row(bd[col0 : col0 + CHUNK])
                    )
                    absorb2(i_bsl)

                    for t in range(nbt):
                        ps = ps_g.tile(
                            [P, CHUNK], fp32, tag="pg", name=f"pg{gi}_{c}_{t}"
                        )
                        nc.tensor.matmul(ps, ident_r, bsl, start=True, stop=False)
                        for kb in range(NKB2):
                            lhsT = (
                                xsT_all[:, t, kb, :]
                                if kb < NKB1
                                else hxT_all[:, t, kb - NKB1, :]
                            )
                            nc.tensor.matmul(
                                ps,
                                lhsT,
                                wch[:, kb, :],
                                start=False,
                                stop=(kb == NKB2 - 1),
                            )
                        nc.vector.bn_stats(stats[t][:, c, :], ps)
                        if c < NCH_G - 1:
                            nc.scalar.copy(zst[t][:, c * CHUNK : (c + 1) * CHUNK], ps)
                        else:
                            # LN scalars: rstd = rsqrt(var+eps), negmu = -mu
                            mv_t = sm2_pool.tile([P, 2], fp32, tag="mv")
                            nc.vector.bn_aggr(mv_t, stats[t])
                            veps_t = sm2_pool.tile([P, 1], fp32, tag="veps")
                            nc.vector.tensor_scalar_add(veps_t, mv_t[:, 1:2], LN_EPS)
                            rstd_t = rsqrt_dve(sm2_pool, veps_t, 2, "rs2")
                            nmu_t = sm2_pool.tile([P, 1], fp32, tag="nmu")
                            nc.vector.tensor_scalar_mul(nmu_t, mv_t[:, 0:1], -1.0)

                            # u = (z - mu) * gamma ; w = u * rstd + beta (in place)
                            u_t = u_pool.tile([P, DIM_H], fp32, tag="u", name=f"u{gi}_{t}")
                            nc.vector.scalar_tensor_tensor(
                                out=u_t[:, 0 : 3 * CHUNK],
                                in0=zst[t],
                                scalar=nmu_t,
                                in1=grep_t[:, 0 : 3 * CHUNK],
                                op0=OP.add,
                                op1=OP.mult,
                            )
                            nc.vector.scalar_tensor_tensor(
                                out=u_t[:, 3 * CHUNK : DIM_H],
                                in0=ps,
                                scalar=nmu_t,
                                in1=grep_t[:, 3 * CHUNK : DIM_H],
                                op0=OP.add,
                                op1=OP.mult,
                            )
                            nc.vector.scalar_tensor_tensor(
                                out=u_t,
                                in0=u_t,
                                scalar=rstd_t,
                                in1=brep_t,
                                op0=OP.mult,
                                op1=OP.add,
                            )
                            if role == "i":
                                nc.scalar.activation(iact[t], u_t, func)
                            else:
                                nc.scalar.activation(u_t, u_t, func)
                                ga = u_t

                            if role == "g":
                                nc.gpsimd.tensor_tensor(iact[t], iact[t], ga, OP.mult)
                            elif role == "f":
                                cx_t = cx_pool.tile(
                                    [P, DIM_H], fp32, tag="cx", name=f"cx{t}"
                                )
                                nc.sync.dma_start(
                                    out=cx_t, in_=cxd[t * P : (t + 1) * P, :]
                                )
                                nc.gpsimd.tensor_tensor(cx_t, ga, cx_t, OP.mult)
                                nc.gpsimd.tensor_tensor(iact[t], iact[t], cx_t, OP.add)
                                nc.scalar.dma_start(
                                    out=cxo[t * P : (t + 1) * P, :], in_=iact[t]
                                )
                            elif role == "o":
                                tnh_t = tnh_pool.tile(
                                    [P, DIM_H], fp32, tag="tnh", name=f"tnh{t}"
                                )
                                nc.scalar.activation(tnh_t, iact[t], AF.Tanh)
                                # hx_new in place of tanh(cx_new)
                                hxn_t = tnh_t
                                nc.gpsimd.tensor_tensor(hxn_t, ga, tnh_t, OP.mult)

                                # second cosine gate
                                dot2 = sm2_pool.tile([P, 1], fp32, tag="dot2")
                                dmp = dv_pool.tile(
                                    [P, DIM_H], fp32, tag="dmp", name=f"dmp{t}"
                                )
                                nc.vector.scalar_tensor_tensor(
                                    out=dmp,
                                    in0=hxn_t,
                                    scalar=1.0,
                                    in1=iact[t],
                                    op0=OP.mult,
                                    op1=OP.mult,
                                    accum_out=dot2,
                                )
                                sq1 = sm2_pool.tile([P, 1], fp32, tag="sq1")
                                dmpa = ps_ad.tile(
                                    [P, DIM_H], fp32, tag="dmpa", name=f"dmpa{t}"
                                )
                                nc.scalar.activation(
                                    dmpa, hxn_t, AF.Square, accum_out=sq1
                                )
                                sq2 = sm2_pool.tile([P, 1], fp32, tag="sq2")
                                dmpb = ps_ad.tile(
                                    [P, DIM_H], fp32, tag="dmpa", name=f"dmpb{t}"
                                )
                                nc.scalar.activation(
                                    dmpb, iact[t], AF.Square, accum_out=sq2
                                )
                                ma = sm2_pool.tile([P, 1], fp32, tag="ma")
                                nc.vector.tensor_scalar_max(ma, sq1, COS_EPS2)
                                mb = sm2_pool.tile([P, 1], fp32, tag="mb")
                                nc.vector.tensor_scalar_max(mb, sq2, COS_EPS2)
                                dn2 = sm2_pool.tile([P, 1], fp32, tag="dn2")
                                nc.vector.tensor_tensor(dn2, ma, mb, OP.mult)
                                rr2 = rsqrt_dve(sm2_pool, dn2, 2, "rs3")
                                arg2 = sm2_pool.tile([P, 1], fp32, tag="arg2")
                                nc.vector.tensor_scalar(
                                    arg2, dot2, rr2, 0.5, OP.mult, OP.mult
                                )
                                co_t = sm2_pool.tile([P, 1], fp32, tag="co")
                                nc.scalar.activation(
                                    co_t, arg2, AF.Sigmoid, bias=halfc
                                )
                                nc.vector.tensor_scalar_add(co_t, co_t, 1.0)
                                nc.vector.tensor_scalar_mul(hxn_t, hxn_t, co_t)
                                nc.scalar.dma_start(
                                    out=hxo[t * P : (t + 1) * P, :], in_=hxn_t
                                )
    if split_waits:
        _split_excess_waits(nc)
    return nc


def _split_excess_waits(nc):
    """Walrus ISA structs have limited sync-wait slots (Matmult/LDW: 1,
    DMA: 2, several DVE/ACT structs: 1-2). The Tile scheduler can emit more.
    Move excess waits onto standalone EventSemaphore instructions injected
    just before the offender on the same engine."""
    import concourse.mybir as mybir

    caps = {}
    skip = {"EventSemaphore", "RegisterMove", "UnconditionalBranch"}
    n_split = 0
    for fn in nc.m.functions:
        for blk in fn.blocks:
            out = []
            changed = False
            for ins in blk.instructions:
                si = ins.sync_info
                opname = type(ins).__name__.replace("Inst", "", 1)
                if (
                    si is not None
                    and si.on_wait
                    and opname not in skip
                    and len(si.on_wait) > caps.get(opname, 1)
                ):
                    cap = caps.get(opname, 1)
                    waits = list(si.on_wait)
                    excess, keep = waits[:-cap], waits[-cap:]
                    for k, w in enumerate(excess):
                        ev = mybir.InstEventSemaphore(
                            name=f"{ins.name}-wsp{k}",
                            ins=[],
                            outs=[],
                            sync_info=mybir.SyncInfo(on_wait=[w], on_update=[]),
                        )
                        ev.engine = ins.engine
                        out.append(ev)
                        n_split += 1
                    ins.sync_info = mybir.SyncInfo(
                        on_wait=keep, on_update=list(si.on_update)
                    )
                    changed = True
                out.append(ins)
            if changed:
                blk.instructions = out
    return n_split


def _get_nc():
    if "nc" not in _cache:
        _cache["nc"] = build_nc()
    return _cache["nc"]


def kernel(x, hx, cx, W, b, Wm, bm, gammas, betas):
    from concourse.bass_utils import run_bass_kernel_spmd

    nc = _get_nc()
    x = np.ascontiguousarray(np.asarray(x, np.float32))
    hx = np.ascontiguousarray(np.asarray(hx, np.float32))
    cx = np.ascontiguousarray(np.asarray(cx, np.float32))
    shared = {
        "W": np.ascontiguousarray(np.asarray(W, np.float32)),
        "b": np.ascontiguousarray(np.asarray(b, np.float32)),
        "Wm": np.ascontiguousarray(np.asarray(Wm, np.float32)),
        "bm": np.ascontiguousarray(np.asarray(bm, np.float32)),
        "gammas": np.ascontiguousarray(np.asarray(gammas, np.float32)),
        "betas": np.ascontiguousarray(np.asarray(betas, np.float32)),
    }
    in_maps = []
    for i in range(NCORES):
        sl = slice(i * BL, (i + 1) * BL)
        in_maps.append({"x": x[sl], "hx": hx[sl], "cx": cx[sl], **shared})
    res = run_bass_kernel_spmd(nc, in_maps, list(range(NCORES)))
    hx_mod = np.concatenate([r["hx_out"] for r in res.results], axis=0)
    cx_new = np.concatenate([r["cx_out"] for r in res.results], axis=0)
    return (hx_mod, cx_new)


# revision 12
# speedup vs baseline: 2.4384x; 1.0133x over previous
"""Trainium2 Bass kernel for the cosine-gated LSTM cell (CGLSTMCellv1).

Full inputs in, full outputs out. Internally: data-parallel shard of the
batch dim across 8 NeuronCores, weights replicated, no cross-core comms.

Math per core (rows = local batch slice):
  mapped = x @ Wm + bm
  attn   = sigmoid(cos_sim(mapped, hx));  s = 1 + attn
  gates  = concat(s*x, hx) @ W + b  = s*(x@Wx) + hx@Wh + b  (s folded into xT)
  i,f,g,o = LN-gates -> sigmoid/tanh
  cx_new = f*cx + i*g ; hx_new = o*tanh(cx_new)
  hx_mod = hx_new * (1 + sigmoid((cos_sim(hx_new,cx_new)+1)/2))

Perf notes vs the fp32 baseline:
  - All GEMM matmuls run as float32r (1 cycle/row when N>=256, vs 4 for
    fp32) via AP.bitcast; PSUM accumulation stays fp32.
  - No ACT sqrt anywhere: rsqrt is a Quake-style bit hack + Newton steps
    on the Vector engine, so the ScalarE activation table stays on the
    sigmoid_and_others set (sigmoid/tanh/square/copy) the whole kernel —
    the baseline burned ~100us in ACT_TABLE_LOADs alternating sqrt<->
    sigmoid.
  - LayerNorm tail consumes PSUM directly: bn_stats on the psum chunks,
    final chunk never copied; apply is two fused scalar_tensor_tensor
    passes  u=(z-mu)*gamma ; w=u*rstd+beta  on DVE, activation on ACT.
  - GpSimd (no PSUM port) only does SBUF-side elementwise (gate combine).

Walrus codegen limits sync waits per instruction (Matmult: 1, DMA: 2), so
PSUM consumers are kept few, and dummy "absorber" transposes pre-observe
DMA semaphores; _split_excess_waits catches the rest.
"""

import numpy as np

B_FULL, DIM_I, DIM_H = 8192, 1024, 1024
NCORES = 8
BL = B_FULL // NCORES  # 1024 rows per core
P = 128
H4 = 4 * DIM_H
NKB1 = DIM_I // P            # 8  k-blocks for mm1
NKB2 = (DIM_I + DIM_H) // P  # 16 k-blocks for mm2
CHUNK = 256                  # W column chunk
NCH_G = DIM_H // CHUNK       # 4 chunks per gate
LN_EPS = 1e-5
COS_EPS2 = 1e-12
QMAGIC = 0x5F3759DF

_cache = {}


def build_nc(nbt=BL // P, split_waits=True):
    """Build the single-core Bass module; nbt = number of 128-row batch tiles."""
    from contextlib import ExitStack

    import concourse.bass as bass
    import concourse.mybir as mybir
    import concourse.tile as tile
    import concourse.tile_rust as tile_rust
    from concourse.masks import make_identity

    fp32 = mybir.dt.float32
    fp32r = mybir.dt.float32r
    i32 = mybir.dt.int32
    AF = mybir.ActivationFunctionType
    OP = mybir.AluOpType
    bl = nbt * P

    def R(ap):
        return ap.bitcast(fp32r)

    nc = bass.Bass()
    xd = nc.dram_tensor("x", [bl, DIM_I], fp32, kind="ExternalInput")
    hxd = nc.dram_tensor("hx", [bl, DIM_H], fp32, kind="ExternalInput")
    cxd = nc.dram_tensor("cx", [bl, DIM_H], fp32, kind="ExternalInput")
    Wd = nc.dram_tensor("W", [DIM_I + DIM_H, H4], fp32r, kind="ExternalInput")
    bd = nc.dram_tensor("b", [H4], fp32r, kind="ExternalInput")
    Wmd = nc.dram_tensor("Wm", [DIM_I, DIM_H], fp32r, kind="ExternalInput")
    bmd = nc.dram_tensor("bm", [DIM_H], fp32r, kind="ExternalInput")
    gd = nc.dram_tensor("gammas", [4, DIM_H], fp32, kind="ExternalInput")
    btd = nc.dram_tensor("betas", [4, DIM_H], fp32, kind="ExternalInput")
    hxo = nc.dram_tensor("hx_out", [bl, DIM_H], fp32, kind="ExternalOutput")
    cxo = nc.dram_tensor("cx_out", [bl, DIM_H], fp32, kind="ExternalOutput")

    def bcast_row(src_ap):
        # view an [N]-shaped AP as [P, N] with 0-step partition broadcast
        return bass.AP(
            tensor=src_ap.tensor, offset=src_ap.offset, ap=[[0, P]] + list(src_ap.ap)
        )

    def raw(inst):
        return getattr(inst, "ins", inst)

    with tile.TileContext(nc) as tc, ExitStack() as ctx:
        singles = ctx.enter_context(tc.tile_pool(name="singles", bufs=1))

        ident = singles.tile([P, P], fp32)
        make_identity(nc, ident)
        ident_r = singles.tile([P, P], fp32r)
        nc.scalar.copy(ident_r, ident)
        ones128 = singles.tile([P, P], fp32)
        nc.vector.memset(ones128, 1.0)
        zrow = singles.tile([P, P], fp32)
        nc.vector.memset(zrow, 0.0)
        halfc = singles.tile([P, 1], fp32)
        nc.vector.memset(halfc, 0.5)
        one_i = singles.tile([P, 1], i32)
        nc.vector.memset(one_i, 1)
        magic_i = singles.tile([P, 1], i32)
        i_msl = nc.vector.memset(magic_i, QMAGIC)

        # transposed activations, persistent across both phases
        xsT_all = singles.tile([P, nbt, NKB1, P], fp32r)
        hxT_all = singles.tile([P, nbt, NKB1, P], fp32r)

        Wv = Wd[:].rearrange("(kb p) n -> p kb n", p=P)
        Wmv = Wmd[:].rearrange("(kb p) n -> p kb n", p=P)

        def colbrd(src, n):
            # view a [P,1] tile as [P,n] with 0-stride free broadcast
            ap = src[:, 0:1]
            return bass.AP(
                tensor=ap.tensor, offset=ap.offset, ap=[list(ap.ap[0]), [0, n]]
            )

        def rsqrt_dve(pool, v_ap, iters, tag):
            """1/sqrt(v) on DVE: Quake bit hack + `iters` Newton steps.
            v_ap: [P,n] fp32 AP. Returns a [P,n] fp32 tile."""
            n = v_ap.free_size()
            vi = v_ap.bitcast(i32)
            y = pool.tile([P, n], fp32, tag=f"{tag}_y")
            yi = y.bitcast(i32)
            t0 = pool.tile([P, n], i32, tag=f"{tag}_t0")
            nc.vector.tensor_tensor(t0, vi, colbrd(one_i, n), OP.logical_shift_right)
            nc.vector.tensor_tensor(yi, colbrd(magic_i, n), t0, OP.subtract)
            for _ in range(iters):
                a = pool.tile([P, n], fp32, tag=f"{tag}_a")
                nc.vector.tensor_tensor(a, v_ap, y, OP.mult)
                nc.vector.tensor_tensor(a, a, y, OP.mult)
                nc.vector.tensor_scalar(a, a, -0.5, 1.5, OP.mult, OP.add)
                nc.vector.tensor_tensor(y, y, a, OP.mult)
            return y

        def absorber(ps_tile):
            def absorb(dep_inst=None):
                """Dummy PE transpose pre-observing one semaphore so real
                matmuls never need more than one sync wait (S3_LW limit)."""
                di = nc.tensor.transpose(ps_tile, ident, ident)
                if dep_inst is not None:
                    tile_rust.add_dep_helper(
                        raw(di), raw(dep_inst), reason="absorb sem for PE"
                    )
                return di

            return absorb

        # ---------------- phase 1 ----------------
        with ExitStack() as p1:
            wm_pool = p1.enter_context(tc.tile_pool(name="wm", bufs=1))
            io_pool = p1.enter_context(tc.tile_pool(name="io1", bufs=2))
            sm_pool = p1.enter_context(tc.tile_pool(name="smalls1", bufs=4))
            dump_pool = p1.enter_context(tc.tile_pool(name="dump1", bufs=3))
            ps_tr = p1.enter_context(tc.tile_pool(name="pstr", bufs=2, space="PSUM"))
            ps_sm = p1.enter_context(tc.tile_pool(name="pssm", bufs=1, space="PSUM"))
            ps_m1 = p1.enter_context(tc.tile_pool(name="psm1", bufs=2, space="PSUM"))

            bm_rep = wm_pool.tile([P, DIM_H], fp32r)
            i_bm = nc.sync.dma_start(out=bm_rep, in_=bcast_row(bmd[:]))
            wm_sb = wm_pool.tile([P, NKB1, DIM_H], fp32r)
            i_wm = nc.sync.dma_start(out=wm_sb, in_=Wmv)

            x_all = wm_pool.tile([P, nbt, DIM_I], fp32)
            hx_all = wm_pool.tile([P, nbt, DIM_H], fp32)
            xload, hxload = [], []
            for t in range(nbt):
                xload.append(
                    nc.sync.dma_start(
                        out=x_all[:, t], in_=xd[t * P : (t + 1) * P, :]
                    )
                )
                hxload.append(
                    nc.sync.dma_start(
                        out=hx_all[:, t], in_=hxd[t * P : (t + 1) * P, :]
                    )
                )

            dmy = ps_sm.tile([P, P], fp32, tag="dmy", bufs=1, name="dmy")
            absorb = absorber(dmy)
            absorb()  # ident (gpsimd sem)
            absorb(i_msl)  # vector memsets
            absorb(i_bm)  # bm_rep dma queue
            absorb(i_wm)  # wm dma queue

            cp_insts = []
            for t in range(nbt):
                x_t = x_all[:, t]
                hx_t = hx_all[:, t]

                absorb(xload[t])
                xT_t = io_pool.tile([P, NKB1, P], fp32r, tag="xT_t")
                for h in range(2):
                    pt = ps_tr.tile([P, 512], fp32, tag="tr", name=f"ptx{t}_{h}")
                    for j in range(4):
                        jj = h * 4 + j
                        nc.tensor.transpose(
                            pt[:, j * P : (j + 1) * P],
                            x_t[:, jj * P : (jj + 1) * P],
                            ident,
                        )
                    nc.scalar.copy(xT_t[:, h * 4 : (h + 1) * 4, :], pt)
                absorb(hxload[t])
                for h in range(2):
                    pt = ps_tr.tile([P, 512], fp32, tag="tr", name=f"pth{t}_{h}")
                    for j in range(4):
                        jj = h * 4 + j
                        nc.tensor.transpose(
                            pt[:, j * P : (j + 1) * P],
                            hx_t[:, jj * P : (jj + 1) * P],
                            ident,
                        )
                    nc.scalar.copy(hxT_all[:, t, h * 4 : (h + 1) * 4, :], pt)

                # mm1: mapped = bm + x @ Wm   (psum [P, 1024], two N=512 groups)
                if t >= 2:
                    for ci in cp_insts[t - 2]:
                        absorb(ci)  # pm slot release (DVE dot + ACT square)
                pm = ps_m1.tile([P, DIM_H], fp32, tag="pm1", name=f"pm{t}")
                for nh in range(2):
                    cs = slice(nh * 512, (nh + 1) * 512)
                    nc.tensor.matmul(
                        pm[:, cs], ident_r, bm_rep[:, cs], start=True, stop=False
                    )
                    for kb in range(NKB1):
                        nc.tensor.matmul(
                            pm[:, cs],
                            xT_t[:, kb, :],
                            wm_sb[:, kb, cs],
                            start=False,
                            stop=(kb == NKB1 - 1),
                        )

                # cosine attention gate; DVE dot + ACT square read the psum
                dot_t = sm_pool.tile([P, 1], fp32, tag="dot")
                dmp0 = dump_pool.tile([P, DIM_H], fp32, tag="dump")
                i_dot = nc.vector.scalar_tensor_tensor(
                    out=dmp0,
                    in0=pm,
                    scalar=1.0,
                    in1=hx_t,
                    op0=OP.mult,
                    op1=OP.mult,
                    accum_out=dot_t,
                )
                sqm_t = sm_pool.tile([P, 1], fp32, tag="sqm")
                dmp1 = dump_pool.tile([P, DIM_H], fp32, tag="dump")
                i_sqm = nc.scalar.activation(dmp1, pm, AF.Square, accum_out=sqm_t)
                cp_insts.append((i_dot, i_sqm))
                sqh_t = sm_pool.tile([P, 1], fp32, tag="sqh")
                dmp2 = dump_pool.tile([P, DIM_H], fp32, tag="dump")
                nc.scalar.activation(dmp2, hx_t, AF.Square, accum_out=sqh_t)

                m1_t = sm_pool.tile([P, 1], fp32, tag="m1")
                nc.vector.tensor_scalar_max(m1_t, sqm_t, COS_EPS2)
                m2_t = sm_pool.tile([P, 1], fp32, tag="m2")
                nc.vector.tensor_scalar_max(m2_t, sqh_t, COS_EPS2)
                den_t = sm_pool.tile([P, 1], fp32, tag="den")
                nc.vector.tensor_tensor(den_t, m1_t, m2_t, OP.mult)
                rinv_t = rsqrt_dve(sm_pool, den_t, 1, "rs1")
                cos_t = sm_pool.tile([P, 1], fp32, tag="cos")
                nc.vector.tensor_scalar_mul(cos_t, dot_t, rinv_t)
                attn_t = sm_pool.tile([P, 1], fp32, tag="attn")
                nc.scalar.activation(attn_t, cos_t, AF.Sigmoid)

                # transpose attn -> row 0 of zrow, replicate via ones-matmul
                psT = ps_sm.tile([1, P], fp32, tag="paux", name=f"psT{t}")
                nc.tensor.transpose(psT, attn_t, ident)
                nc.scalar.copy(zrow[0:1, :], psT)
                psr = ps_sm.tile([P, P], fp32, tag="paux", name=f"psr{t}")
                nc.tensor.matmul(psr, ones128, zrow, start=True, stop=True)
                srep_t = sm_pool.tile([P, P], fp32, tag="srep")
                nc.scalar.copy(srep_t, psr)

                srep_brd = bass.AP(
                    tensor=srep_t.tensor,
                    offset=srep_t.offset,
                    ap=[list(srep_t.ap[0]), [0, NKB1], list(srep_t.ap[1])],
                )
                # xsT = (1 + attn) * xT in one DVE pass
                nc.vector.scalar_tensor_tensor(
                    out=xsT_all[:, t],
                    in0=srep_brd,
                    scalar=1.0,
                    in1=xT_t,
                    op0=OP.add,
                    op1=OP.mult,
                )


        # ---------------- phase 2 ----------------
        with ExitStack() as p2:
            w_pool = p2.enter_context(tc.tile_pool(name="wch", bufs=2))
            bsl_pool = p2.enter_context(tc.tile_pool(name="bsl", bufs=2))
            gb_pool = p2.enter_context(tc.tile_pool(name="gb", bufs=1))
            iact_pool = p2.enter_context(tc.tile_pool(name="iact", bufs=nbt))
            zst_pool = p2.enter_context(tc.tile_pool(name="zst", bufs=nbt))
            u_pool = p2.enter_context(tc.tile_pool(name="u", bufs=3))
            tnh_pool = p2.enter_context(tc.tile_pool(name="tnh", bufs=1))
            st_pool = p2.enter_context(tc.tile_pool(name="stats", bufs=nbt + 2))
            v_pool = p2.enter_context(tc.tile_pool(name="vall", bufs=2))
            cx_pool = p2.enter_context(tc.tile_pool(name="cxin", bufs=2))
            dv_pool = p2.enter_context(tc.tile_pool(name="dvdump", bufs=2))
            sm2_pool = p2.enter_context(tc.tile_pool(name="smalls2", bufs=2))
            ps_g = p2.enter_context(tc.tile_pool(name="psg", bufs=5, space="PSUM"))
            ps_ad = p2.enter_context(
                tc.tile_pool(name="psact", bufs=1, space="PSUM")
            )

            dmy2 = ps_g.tile([P, P], fp32, tag="dmy2", bufs=1, name="dmy2")
            absorb2 = absorber(dmy2)

            iact = [
                iact_pool.tile([P, DIM_H], fp32, tag="iact", name=f"iact{t}")
                for t in range(nbt)
            ]
            zst = [
                zst_pool.tile([P, DIM_H], fp32, tag="zst", name=f"zst{t}")
                for t in range(nbt)
            ]
            stats = [
                st_pool.tile([P, NCH_G, 6], fp32, tag="st", name=f"st{t}")
                for t in range(nbt)
            ]

            # gate order: i first (stored), then g (i*g), f (cx_new), o (outputs)
            for gi, func, role in (
                (0, AF.Sigmoid, "i"),
                (2, AF.Tanh, "g"),
                (1, AF.Sigmoid, "f"),
                (3, AF.Sigmoid, "o"),
            ):
                grep_t = gb_pool.tile([P, DIM_H], fp32, tag="grep", name=f"grep{gi}")
                nc.sync.dma_start(out=grep_t, in_=bcast_row(gd[gi, :]))
                brep_t = gb_pool.tile([P, DIM_H], fp32, tag="brep", name=f"brep{gi}")
                nc.sync.dma_start(out=brep_t, in_=bcast_row(btd[gi, :]))
                vall = v_pool.tile([P, nbt, 2], fp32, tag="vall", name=f"vall{gi}")

                for c in range(NCH_G):
                    col0 = gi * DIM_H + c * CHUNK
                    wch = w_pool.tile(
                        [P, NKB2, CHUNK], fp32r, tag="wch", name=f"wch{gi}_{c}"
                    )
                    nc.sync.dma_start(out=wch, in_=Wv[:, :, col0 : col0 + CHUNK])
                    bsl = bsl_pool.tile(
                        [P, CHUNK], fp32r, tag="bsl", name=f"bsl{gi}_{c}"
                    )
                    i_bsl = nc.sync.dma_start(
                        out=bsl, in_=bcast_row(bd[col0 : col0 + CHUNK])
                    )
                    absorb2(i_bsl)

                    for t in range(nbt):
                        ps = ps_g.tile(
                            [P, CHUNK], fp32, tag="pg", name=f"pg{gi}_{c}_{t}"
                        )
                        nc.tensor.matmul(ps, ident_r, bsl, start=True, stop=False)
                        for kb in range(NKB2):
                            lhsT = (
                                xsT_all[:, t, kb, :]
                                if kb < NKB1
                                else hxT_all[:, t, kb - NKB1, :]
                            )
                            nc.tensor.matmul(
                                ps,
                                lhsT,
                                wch[:, kb, :],
                                start=False,
                                stop=(kb == NKB2 - 1),
                            )
                        nc.vector.bn_stats(stats[t][:, c, :], ps)
                        nc.scalar.copy(zst[t][:, c * CHUNK : (c + 1) * CHUNK], ps)
                        if c == NCH_G - 1:
                            nc.vector.bn_aggr(vall[:, t, :], stats[t])

                # batched LN scalars for all tiles of this gate
                veps_t = sm2_pool.tile([P, nbt], fp32, tag="veps")
                nc.vector.tensor_scalar_add(veps_t, vall[:, :, 1:2], LN_EPS)
                rstd_all = rsqrt_dve(sm2_pool, veps_t, 2, "rs2")
                nmu_all = sm2_pool.tile([P, nbt], fp32, tag="nmu")
                nc.vector.tensor_scalar_mul(nmu_all, vall[:, :, 0:1], -1.0)

                for t in range(nbt):
                    # u = (z - mu) * gamma ; w = u * rstd + beta (in place)
                    u_t = u_pool.tile([P, DIM_H], fp32, tag="u", name=f"u{gi}_{t}")
                    nc.vector.scalar_tensor_tensor(
                        out=u_t,
                        in0=zst[t],
                        scalar=nmu_all[:, t : t + 1],
                        in1=grep_t,
                        op0=OP.add,
                        op1=OP.mult,
                    )
                    nc.vector.scalar_tensor_tensor(
                        out=u_t,
                        in0=u_t,
                        scalar=rstd_all[:, t : t + 1],
                        in1=brep_t,
                        op0=OP.mult,
                        op1=OP.add,
                    )
                    if role == "i":
                        nc.scalar.activation(iact[t], u_t, func)
                    else:
                        nc.scalar.activation(u_t, u_t, func)
                        ga = u_t

                    if role == "g":
                        nc.gpsimd.tensor_tensor(iact[t], iact[t], ga, OP.mult)
                    elif role == "f":
                        cx_t = cx_pool.tile(
                            [P, DIM_H], fp32, tag="cx", name=f"cx{t}"
                        )
                        nc.sync.dma_start(
                            out=cx_t, in_=cxd[t * P : (t + 1) * P, :]
                        )
                        nc.gpsimd.tensor_tensor(cx_t, ga, cx_t, OP.mult)
                        # iact += ga*cx on DVE (keeps the o-tail Pool-light)
                        nc.vector.scalar_tensor_tensor(
                            out=iact[t],
                            in0=cx_t,
                            scalar=1.0,
                            in1=iact[t],
                            op0=OP.mult,
                            op1=OP.add,
                        )
                        nc.scalar.dma_start(
                            out=cxo[t * P : (t + 1) * P, :], in_=iact[t]
                        )
                    elif role == "o":
                        tnh_t = tnh_pool.tile(
                            [P, DIM_H], fp32, tag="tnh", name=f"tnh{t}"
                        )
                        nc.scalar.activation(tnh_t, iact[t], AF.Tanh)
                        # hx_new in place of tanh(cx_new)
                        hxn_t = tnh_t
                        nc.gpsimd.tensor_tensor(hxn_t, ga, tnh_t, OP.mult)

                        # second cosine gate
                        dot2 = sm2_pool.tile([P, 1], fp32, tag="dot2")
                        dmp = dv_pool.tile(
                            [P, DIM_H], fp32, tag="dmp", name=f"dmp{t}"
                        )
                        nc.vector.scalar_tensor_tensor(
                            out=dmp,
                            in0=hxn_t,
                            scalar=1.0,
                            in1=iact[t],
                            op0=OP.mult,
                            op1=OP.mult,
                            accum_out=dot2,
                        )
                        sq1 = sm2_pool.tile([P, 1], fp32, tag="sq1")
                        dmpa = ps_ad.tile(
                            [P, DIM_H], fp32, tag="dmpa", name=f"dmpa{t}"
                        )
                        nc.scalar.activation(
                            dmpa, hxn_t, AF.Square, accum_out=sq1
                        )
                        sq2 = sm2_pool.tile([P, 1], fp32, tag="sq2")
                        dmpb = ps_ad.tile(
                            [P, DIM_H], fp32, tag="dmpa", name=f"dmpb{t}"
                        )
                        nc.scalar.activation(
                            dmpb, iact[t], AF.Square, accum_out=sq2
                        )
                        ma = sm2_pool.tile([P, 1], fp32, tag="ma")
                        nc.vector.tensor_scalar_max(ma, sq1, COS_EPS2)
                        mb = sm2_pool.tile([P, 1], fp32, tag="mb")
                        nc.vector.tensor_scalar_max(mb, sq2, COS_EPS2)
                        dn2 = sm2_pool.tile([P, 1], fp32, tag="dn2")
                        nc.vector.tensor_tensor(dn2, ma, mb, OP.mult)
                        rr2 = rsqrt_dve(sm2_pool, dn2, 1, "rs3")
                        arg2 = sm2_pool.tile([P, 1], fp32, tag="arg2")
                        nc.vector.tensor_scalar(
                            arg2, dot2, rr2, 0.5, OP.mult, OP.mult
                        )
                        co_t = sm2_pool.tile([P, 1], fp32, tag="co")
                        nc.scalar.activation(
                            co_t, arg2, AF.Sigmoid, bias=halfc
                        )
                        # hx_mod = hxn*co + hxn in one DVE pass
                        nc.vector.scalar_tensor_tensor(
                            out=hxn_t,
                            in0=hxn_t,
                            scalar=co_t,
                            in1=hxn_t,
                            op0=OP.mult,
                            op1=OP.add,
                        )
                        nc.scalar.dma_start(
                            out=hxo[t * P : (t + 1) * P, :], in_=hxn_t
                        )
    if split_waits:
        _split_excess_waits(nc)
    return nc


def _split_excess_waits(nc):
    """Walrus ISA structs have limited sync-wait slots (Matmult/LDW: 1,
    DMA: 2, several DVE/ACT structs: 1-2). The Tile scheduler can emit more.
    Move excess waits onto standalone EventSemaphore instructions injected
    just before the offender on the same engine."""
    import concourse.mybir as mybir

    caps = {}
    skip = {"EventSemaphore", "RegisterMove", "UnconditionalBranch"}
    n_split = 0
    for fn in nc.m.functions:
        for blk in fn.blocks:
            out = []
            changed = False
            for ins in blk.instructions:
                si = ins.sync_info
                opname = type(ins).__name__.replace("Inst", "", 1)
                if (
                    si is not None
                    and si.on_wait
                    and opname not in skip
                    and len(si.on_wait) > caps.get(opname, 1)
                ):
                    cap = caps.get(opname, 1)
                    waits = list(si.on_wait)
                    excess, keep = waits[:-cap], waits[-cap:]
                    for k, w in enumerate(excess):
                        ev = mybir.InstEventSemaphore(
                            name=f"{ins.name}-wsp{k}",
                            ins=[],
                            outs=[],
                            sync_info=mybir.SyncInfo(on_wait=[w], on_update=[]),
                        )
                        ev.engine = ins.engine
                        out.append(ev)
                        n_split += 1
                    ins.sync_info = mybir.SyncInfo(
                        on_wait=keep, on_update=list(si.on_update)
                    )
                    changed = True
                out.append(ins)
            if changed:
                blk.instructions = out
    return n_split


def _get_nc():
    if "nc" not in _cache:
        _cache["nc"] = build_nc()
    return _cache["nc"]


def kernel(x, hx, cx, W, b, Wm, bm, gammas, betas):
    from concourse.bass_utils import run_bass_kernel_spmd

    nc = _get_nc()
    x = np.ascontiguousarray(np.asarray(x, np.float32))
    hx = np.ascontiguousarray(np.asarray(hx, np.float32))
    cx = np.ascontiguousarray(np.asarray(cx, np.float32))
    shared = {
        "W": np.ascontiguousarray(np.asarray(W, np.float32)),
        "b": np.ascontiguousarray(np.asarray(b, np.float32)),
        "Wm": np.ascontiguousarray(np.asarray(Wm, np.float32)),
        "bm": np.ascontiguousarray(np.asarray(bm, np.float32)),
        "gammas": np.ascontiguousarray(np.asarray(gammas, np.float32)),
        "betas": np.ascontiguousarray(np.asarray(betas, np.float32)),
    }
    in_maps = []
    for i in range(NCORES):
        sl = slice(i * BL, (i + 1) * BL)
        in_maps.append({"x": x[sl], "hx": hx[sl], "cx": cx[sl], **shared})
    res = run_bass_kernel_spmd(nc, in_maps, list(range(NCORES)))
    hx_mod = np.concatenate([r["hx_out"] for r in res.results], axis=0)
    cx_new = np.concatenate([r["cx_out"] for r in res.results], axis=0)
    return (hx_mod, cx_new)


# revision 14
# speedup vs baseline: 2.5311x; 1.0380x over previous
"""Trainium2 Bass kernel for the cosine-gated LSTM cell (CGLSTMCellv1).

Full inputs in, full outputs out. Internally: data-parallel shard of the
batch dim across 8 NeuronCores, weights replicated, no cross-core comms.

Math per core (rows = local batch slice):
  mapped = x @ Wm + bm
  attn   = sigmoid(cos_sim(mapped, hx));  s = 1 + attn
  gates  = concat(s*x, hx) @ W + b  = s*(x@Wx) + hx@Wh + b  (s folded into xT)
  i,f,g,o = LN-gates -> sigmoid/tanh
  cx_new = f*cx + i*g ; hx_new = o*tanh(cx_new)
  hx_mod = hx_new * (1 + sigmoid((cos_sim(hx_new,cx_new)+1)/2))

Perf notes vs the fp32 baseline:
  - All GEMM matmuls run as float32r (1 cycle/row when N>=256, vs 4 for
    fp32) via AP.bitcast; PSUM accumulation stays fp32.
  - No ACT sqrt anywhere: rsqrt is a Quake-style bit hack + Newton steps
    on the Vector engine, so the ScalarE activation table stays on the
    sigmoid_and_others set (sigmoid/tanh/square/copy) the whole kernel —
    the baseline burned ~100us in ACT_TABLE_LOADs alternating sqrt<->
    sigmoid.
  - LayerNorm tail consumes PSUM directly: bn_stats on the psum chunks,
    final chunk never copied; apply is two fused scalar_tensor_tensor
    passes  u=(z-mu)*gamma ; w=u*rstd+beta  on DVE, activation on ACT.
  - GpSimd (no PSUM port) only does SBUF-side elementwise (gate combine).

Walrus codegen limits sync waits per instruction (Matmult: 1, DMA: 2), so
PSUM consumers are kept few, and dummy "absorber" transposes pre-observe
DMA semaphores; _split_excess_waits catches the rest.
"""

import numpy as np

B_FULL, DIM_I, DIM_H = 8192, 1024, 1024
NCORES = 8
BL = B_FULL // NCORES  # 1024 rows per core
P = 128
H4 = 4 * DIM_H
NKB1 = DIM_I // P            # 8  k-blocks for mm1
NKB2 = (DIM_I + DIM_H) // P  # 16 k-blocks for mm2
CHUNK = 256                  # W column chunk
NCH_G = DIM_H // CHUNK       # 4 chunks per gate
LN_EPS = 1e-5
COS_EPS2 = 1e-12
QMAGIC = 0x5F3759DF

_cache = {}


def build_nc(nbt=BL // P, split_waits=True):
    """Build the single-core Bass module; nbt = number of 128-row batch tiles."""
    from contextlib import ExitStack

    import concourse.bass as bass
    import concourse.mybir as mybir
    import concourse.tile as tile
    import concourse.tile_rust as tile_rust
    from concourse.masks import make_identity

    fp32 = mybir.dt.float32
    fp32r = mybir.dt.float32r
    i32 = mybir.dt.int32
    AF = mybir.ActivationFunctionType
    OP = mybir.AluOpType
    bl = nbt * P

    def R(ap):
        return ap.bitcast(fp32r)

    nc = bass.Bass()
    xd = nc.dram_tensor("x", [bl, DIM_I], fp32, kind="ExternalInput")
    hxd = nc.dram_tensor("hx", [bl, DIM_H], fp32, kind="ExternalInput")
    cxd = nc.dram_tensor("cx", [bl, DIM_H], fp32, kind="ExternalInput")
    Wd = nc.dram_tensor("W", [DIM_I + DIM_H, H4], fp32r, kind="ExternalInput")
    bd = nc.dram_tensor("b", [H4], fp32r, kind="ExternalInput")
    Wmd = nc.dram_tensor("Wm", [DIM_I, DIM_H], fp32r, kind="ExternalInput")
    bmd = nc.dram_tensor("bm", [DIM_H], fp32r, kind="ExternalInput")
    gd = nc.dram_tensor("gammas", [4, DIM_H], fp32, kind="ExternalInput")
    btd = nc.dram_tensor("betas", [4, DIM_H], fp32, kind="ExternalInput")
    hxo = nc.dram_tensor("hx_out", [bl, DIM_H], fp32, kind="ExternalOutput")
    cxo = nc.dram_tensor("cx_out", [bl, DIM_H], fp32, kind="ExternalOutput")

    def bcast_row(src_ap):
        # view an [N]-shaped AP as [P, N] with 0-step partition broadcast
        return bass.AP(
            tensor=src_ap.tensor, offset=src_ap.offset, ap=[[0, P]] + list(src_ap.ap)
        )

    def raw(inst):
        return getattr(inst, "ins", inst)

    with tile.TileContext(nc) as tc, ExitStack() as ctx:
        singles = ctx.enter_context(tc.tile_pool(name="singles", bufs=1))

        ident = singles.tile([P, P], fp32)
        make_identity(nc, ident)
        ident_r = singles.tile([P, P], fp32r)
        nc.scalar.copy(ident_r, ident)
        ones128 = singles.tile([P, P], fp32)
        nc.vector.memset(ones128, 1.0)
        zrow = singles.tile([P, P], fp32)
        nc.vector.memset(zrow, 0.0)
        halfc = singles.tile([P, 1], fp32)
        nc.vector.memset(halfc, 0.5)
        one_i = singles.tile([P, 1], i32)
        nc.vector.memset(one_i, 1)
        magic_i = singles.tile([P, 1], i32)
        i_msl = nc.vector.memset(magic_i, QMAGIC)

        # transposed activations, persistent across both phases
        xsT_all = singles.tile([P, nbt, NKB1, P], fp32r)
        hxT_all = singles.tile([P, nbt, NKB1, P], fp32r)

        Wv = Wd[:].rearrange("(kb p) n -> p kb n", p=P)
        Wmv = Wmd[:].rearrange("(kb p) n -> p kb n", p=P)

        def colbrd(src, n):
            # view a [P,1] tile as [P,n] with 0-stride free broadcast
            ap = src[:, 0:1]
            return bass.AP(
                tensor=ap.tensor, offset=ap.offset, ap=[list(ap.ap[0]), [0, n]]
            )

        def rsqrt_dve(pool, v_ap, iters, tag):
            """1/sqrt(v) on DVE: Quake bit hack + `iters` Newton steps.
            v_ap: [P,n] fp32 AP. Returns a [P,n] fp32 tile."""
            n = v_ap.free_size()
            vi = v_ap.bitcast(i32)
            y = pool.tile([P, n], fp32, tag=f"{tag}_y")
            yi = y.bitcast(i32)
            t0 = pool.tile([P, n], i32, tag=f"{tag}_t0")
            nc.vector.tensor_tensor(t0, vi, colbrd(one_i, n), OP.logical_shift_right)
            nc.vector.tensor_tensor(yi, colbrd(magic_i, n), t0, OP.subtract)
            for _ in range(iters):
                a = pool.tile([P, n], fp32, tag=f"{tag}_a")
                nc.vector.tensor_tensor(a, v_ap, y, OP.mult)
                nc.vector.tensor_tensor(a, a, y, OP.mult)
                nc.vector.tensor_scalar(a, a, -0.5, 1.5, OP.mult, OP.add)
                nc.vector.tensor_tensor(y, y, a, OP.mult)
            return y

        def absorber(ps_tile):
            def absorb(dep_inst=None):
                """Dummy PE transpose pre-observing one semaphore so real
                matmuls never need more than one sync wait (S3_LW limit)."""
                di = nc.tensor.transpose(ps_tile, ident, ident)
                if dep_inst is not None:
                    tile_rust.add_dep_helper(
                        raw(di), raw(dep_inst), reason="absorb sem for PE"
                    )
                return di

            return absorb

        # ---------------- phase 1 ----------------
        with ExitStack() as p1:
            wm_pool = p1.enter_context(tc.tile_pool(name="wm", bufs=1))
            io_pool = p1.enter_context(tc.tile_pool(name="io1", bufs=2))
            sm_pool = p1.enter_context(tc.tile_pool(name="smalls1", bufs=4))
            dump_pool = p1.enter_context(tc.tile_pool(name="dump1", bufs=3))
            ps_tr = p1.enter_context(tc.tile_pool(name="pstr", bufs=2, space="PSUM"))
            ps_sm = p1.enter_context(tc.tile_pool(name="pssm", bufs=1, space="PSUM"))
            ps_m1 = p1.enter_context(tc.tile_pool(name="psm1", bufs=2, space="PSUM"))

            bm_rep = wm_pool.tile([P, DIM_H], fp32r)
            i_bm = nc.sync.dma_start(out=bm_rep, in_=bcast_row(bmd[:]))
            wm_sb = wm_pool.tile([P, NKB1, DIM_H], fp32r)
            i_wm = nc.sync.dma_start(out=wm_sb, in_=Wmv)

            x_all = wm_pool.tile([P, nbt, DIM_I], fp32)
            hx_all = wm_pool.tile([P, nbt, DIM_H], fp32)
            xload, hxload = [], []
            for t in range(nbt):
                xload.append(
                    nc.sync.dma_start(
                        out=x_all[:, t], in_=xd[t * P : (t + 1) * P, :]
                    )
                )
                hxload.append(
                    nc.scalar.dma_start(
                        out=hx_all[:, t], in_=hxd[t * P : (t + 1) * P, :]
                    )
                )

            dmy = ps_sm.tile([P, P], fp32, tag="dmy", bufs=1, name="dmy")
            absorb = absorber(dmy)
            absorb()  # ident (gpsimd sem)
            absorb(i_msl)  # vector memsets
            absorb(i_bm)  # bm_rep dma queue
            absorb(i_wm)  # wm dma queue

            cp_insts = []
            for t in range(nbt):
                x_t = x_all[:, t]
                hx_t = hx_all[:, t]

                absorb(xload[t])
                xT_t = io_pool.tile([P, NKB1, P], fp32r, tag="xT_t")
                for h in range(2):
                    pt = ps_tr.tile([P, 512], fp32, tag="tr", name=f"ptx{t}_{h}")
                    for j in range(4):
                        jj = h * 4 + j
                        nc.tensor.transpose(
                            pt[:, j * P : (j + 1) * P],
                            x_t[:, jj * P : (jj + 1) * P],
                            ident,
                        )
                    nc.scalar.copy(xT_t[:, h * 4 : (h + 1) * 4, :], pt)
                absorb(hxload[t])
                for h in range(2):
                    pt = ps_tr.tile([P, 512], fp32, tag="tr", name=f"pth{t}_{h}")
                    for j in range(4):
                        jj = h * 4 + j
                        nc.tensor.transpose(
                            pt[:, j * P : (j + 1) * P],
                            hx_t[:, jj * P : (jj + 1) * P],
                            ident,
                        )
                    nc.scalar.copy(hxT_all[:, t, h * 4 : (h + 1) * 4, :], pt)

                # mm1: mapped = bm + x @ Wm   (psum [P, 1024], two N=512 groups)
                if t >= 2:
                    for ci in cp_insts[t - 2]:
                        absorb(ci)  # pm slot release (DVE dot + ACT square)
                pm = ps_m1.tile([P, DIM_H], fp32, tag="pm1", name=f"pm{t}")
                for nh in range(2):
                    cs = slice(nh * 512, (nh + 1) * 512)
                    nc.tensor.matmul(
                        pm[:, cs], ident_r, bm_rep[:, cs], start=True, stop=False
                    )
                    for kb in range(NKB1):
                        nc.tensor.matmul(
                            pm[:, cs],
                            xT_t[:, kb, :],
                            wm_sb[:, kb, cs],
                            start=False,
                            stop=(kb == NKB1 - 1),
                        )

                # cosine attention gate; DVE dot + ACT square read the psum
                dot_t = sm_pool.tile([P, 1], fp32, tag="dot")
                dmp0 = dump_pool.tile([P, DIM_H], fp32, tag="dump")
                i_dot = nc.vector.scalar_tensor_tensor(
                    out=dmp0,
                    in0=pm,
                    scalar=1.0,
                    in1=hx_t,
                    op0=OP.mult,
                    op1=OP.mult,
                    accum_out=dot_t,
                )
                sqm_t = sm_pool.tile([P, 1], fp32, tag="sqm")
                dmp1 = dump_pool.tile([P, DIM_H], fp32, tag="dump")
                i_sqm = nc.scalar.activation(dmp1, pm, AF.Square, accum_out=sqm_t)
                cp_insts.append((i_dot, i_sqm))
                sqh_t = sm_pool.tile([P, 1], fp32, tag="sqh")
                dmp2 = dump_pool.tile([P, DIM_H], fp32, tag="dump")
                nc.scalar.activation(dmp2, hx_t, AF.Square, accum_out=sqh_t)

                m1_t = sm_pool.tile([P, 1], fp32, tag="m1")
                nc.vector.tensor_scalar_max(m1_t, sqm_t, COS_EPS2)
                m2_t = sm_pool.tile([P, 1], fp32, tag="m2")
                nc.vector.tensor_scalar_max(m2_t, sqh_t, COS_EPS2)
                den_t = sm_pool.tile([P, 1], fp32, tag="den")
                nc.vector.tensor_tensor(den_t, m1_t, m2_t, OP.mult)
                rinv_t = rsqrt_dve(sm_pool, den_t, 1, "rs1")
                cos_t = sm_pool.tile([P, 1], fp32, tag="cos")
                nc.vector.tensor_scalar_mul(cos_t, dot_t, rinv_t)
                attn_t = sm_pool.tile([P, 1], fp32, tag="attn")
                nc.scalar.activation(attn_t, cos_t, AF.Sigmoid)

                # transpose attn -> row 0 of zrow, replicate via ones-matmul
                psT = ps_sm.tile([1, P], fp32, tag="paux", name=f"psT{t}")
                nc.tensor.transpose(psT, attn_t, ident)
                nc.scalar.copy(zrow[0:1, :], psT)
                psr = ps_sm.tile([P, P], fp32, tag="paux", name=f"psr{t}")
                nc.tensor.matmul(psr, ones128, zrow, start=True, stop=True)
                srep_t = sm_pool.tile([P, P], fp32, tag="srep")
                nc.scalar.copy(srep_t, psr)

                srep_brd = bass.AP(
                    tensor=srep_t.tensor,
                    offset=srep_t.offset,
                    ap=[list(srep_t.ap[0]), [0, NKB1], list(srep_t.ap[1])],
                )
                # xsT = (1 + attn) * xT in one DVE pass
                nc.vector.scalar_tensor_tensor(
                    out=xsT_all[:, t],
                    in0=srep_brd,
                    scalar=1.0,
                    in1=xT_t,
                    op0=OP.add,
                    op1=OP.mult,
                )


        # ---------------- phase 2 ----------------
        with ExitStack() as p2:
            w_pool = p2.enter_context(tc.tile_pool(name="wch", bufs=2))
            bsl_pool = p2.enter_context(tc.tile_pool(name="bsl", bufs=2))
            gb_pool = p2.enter_context(tc.tile_pool(name="gb", bufs=1))
            iact_pool = p2.enter_context(tc.tile_pool(name="iact", bufs=nbt))
            zst_pool = p2.enter_context(tc.tile_pool(name="zst", bufs=nbt))
            u_pool = p2.enter_context(tc.tile_pool(name="u", bufs=2))
            tnh_pool = p2.enter_context(tc.tile_pool(name="tnh", bufs=2))
            st_pool = p2.enter_context(tc.tile_pool(name="stats", bufs=nbt + 2))
            v_pool = p2.enter_context(tc.tile_pool(name="vall", bufs=2))
            cx_pool = p2.enter_context(tc.tile_pool(name="cxin", bufs=2))
            dv_pool = p2.enter_context(tc.tile_pool(name="dvdump", bufs=2))
            sm2_pool = p2.enter_context(tc.tile_pool(name="smalls2", bufs=2))
            sq2_pool = p2.enter_context(tc.tile_pool(name="sq2p", bufs=nbt))
            ps_g = p2.enter_context(tc.tile_pool(name="psg", bufs=5, space="PSUM"))
            ps_ad = p2.enter_context(
                tc.tile_pool(name="psact", bufs=1, space="PSUM")
            )

            dmy2 = ps_g.tile([P, P], fp32, tag="dmy2", bufs=1, name="dmy2")
            absorb2 = absorber(dmy2)

            iact = [
                iact_pool.tile([P, DIM_H], fp32, tag="iact", name=f"iact{t}")
                for t in range(nbt)
            ]
            zst = [
                zst_pool.tile([P, DIM_H], fp32, tag="zst", name=f"zst{t}")
                for t in range(nbt)
            ]
            stats = [
                st_pool.tile([P, NCH_G, 6], fp32, tag="st", name=f"st{t}")
                for t in range(nbt)
            ]

            sq2s = []
            # gate order: i first (stored), then g (i*g), f (cx_new), o (outputs)
            for gi, func, role in (
                (0, AF.Sigmoid, "i"),
                (2, AF.Tanh, "g"),
                (1, AF.Sigmoid, "f"),
                (3, AF.Sigmoid, "o"),
            ):
                grep_t = gb_pool.tile([P, DIM_H], fp32, tag="grep", name=f"grep{gi}")
                nc.scalar.dma_start(out=grep_t, in_=bcast_row(gd[gi, :]))
                brep_t = gb_pool.tile([P, DIM_H], fp32, tag="brep", name=f"brep{gi}")
                nc.scalar.dma_start(out=brep_t, in_=bcast_row(btd[gi, :]))
                vall = v_pool.tile([P, nbt, 2], fp32, tag="vall", name=f"vall{gi}")

                for c in range(NCH_G):
                    col0 = gi * DIM_H + c * CHUNK
                    wch = w_pool.tile(
                        [P, NKB2, CHUNK], fp32r, tag="wch", name=f"wch{gi}_{c}"
                    )
                    nc.sync.dma_start(out=wch, in_=Wv[:, :, col0 : col0 + CHUNK])
                    bsl = bsl_pool.tile(
                        [P, CHUNK], fp32r, tag="bsl", name=f"bsl{gi}_{c}"
                    )
                    i_bsl = nc.sync.dma_start(
                        out=bsl, in_=bcast_row(bd[col0 : col0 + CHUNK])
                    )
                    absorb2(i_bsl)

                    for t in range(nbt):
                        ps = ps_g.tile(
                            [P, CHUNK], fp32, tag="pg", name=f"pg{gi}_{c}_{t}"
                        )
                        nc.tensor.matmul(ps, ident_r, bsl, start=True, stop=False)
                        for kb in range(NKB2):
                            lhsT = (
                                xsT_all[:, t, kb, :]
                                if kb < NKB1
                                else hxT_all[:, t, kb - NKB1, :]
                            )
                            nc.tensor.matmul(
                                ps,
                                lhsT,
                                wch[:, kb, :],
                                start=False,
                                stop=(kb == NKB2 - 1),
                            )
                        nc.vector.bn_stats(stats[t][:, c, :], ps)
                        nc.scalar.copy(zst[t][:, c * CHUNK : (c + 1) * CHUNK], ps)
                        if c == NCH_G - 1:
                            nc.vector.bn_aggr(vall[:, t, :], stats[t])

                # batched LN scalars for all tiles of this gate
                veps_t = sm2_pool.tile([P, nbt], fp32, tag="veps")
                nc.vector.tensor_scalar_add(veps_t, vall[:, :, 1:2], LN_EPS)
                rstd_all = rsqrt_dve(sm2_pool, veps_t, 2, "rs2")
                nmu_all = sm2_pool.tile([P, nbt], fp32, tag="nmu")
                nc.vector.tensor_scalar_mul(nmu_all, vall[:, :, 0:1], -1.0)

                for t in range(nbt):
                    # u = (z - mu) * gamma ; w = u * rstd + beta (in place)
                    u_t = u_pool.tile([P, DIM_H], fp32, tag="u", name=f"u{gi}_{t}")
                    nc.vector.scalar_tensor_tensor(
                        out=u_t,
                        in0=zst[t],
                        scalar=nmu_all[:, t : t + 1],
                        in1=grep_t,
                        op0=OP.add,
                        op1=OP.mult,
                    )
                    nc.vector.scalar_tensor_tensor(
                        out=u_t,
                        in0=u_t,
                        scalar=rstd_all[:, t : t + 1],
                        in1=brep_t,
                        op0=OP.mult,
                        op1=OP.add,
                    )
                    if role == "i":
                        nc.scalar.activation(iact[t], u_t, func)
                    else:
                        nc.scalar.activation(u_t, u_t, func)
                        ga = u_t

                    if role == "g":
                        nc.gpsimd.tensor_tensor(iact[t], iact[t], ga, OP.mult)
                    elif role == "f":
                        cx_t = cx_pool.tile(
                            [P, DIM_H], fp32, tag="cx", name=f"cx{t}"
                        )
                        nc.sync.dma_start(
                            out=cx_t, in_=cxd[t * P : (t + 1) * P, :]
                        )
                        nc.gpsimd.tensor_tensor(cx_t, ga, cx_t, OP.mult)
                        # iact += ga*cx on DVE (keeps the o-tail Pool-light)
                        nc.vector.scalar_tensor_tensor(
                            out=iact[t],
                            in0=cx_t,
                            scalar=1.0,
                            in1=iact[t],
                            op0=OP.mult,
                            op1=OP.add,
                        )
                        nc.scalar.dma_start(
                            out=cxo[t * P : (t + 1) * P, :], in_=iact[t]
                        )
                        sq2 = sq2_pool.tile([P, 1], fp32, tag="sq2", name=f"sq2_{t}")
                        sq2s.append(sq2)
                        dmpb = ps_ad.tile(
                            [P, DIM_H], fp32, tag="dmpa", name=f"dmpb{t}"
                        )
                        nc.scalar.activation(
                            dmpb, iact[t], AF.Square, accum_out=sq2
                        )
                    elif role == "o":
                        tnh_t = tnh_pool.tile(
                            [P, DIM_H], fp32, tag="tnh", name=f"tnh{t}"
                        )
                        nc.scalar.activation(tnh_t, iact[t], AF.Tanh)
                        # hx_new in place of tanh(cx_new); split halves
                        hxn_t = tnh_t
                        H2 = DIM_H // 2
                        nc.gpsimd.tensor_tensor(
                            hxn_t[:, 0:H2], ga[:, 0:H2], tnh_t[:, 0:H2], OP.mult
                        )
                        nc.vector.tensor_tensor(
                            hxn_t[:, H2:DIM_H], ga[:, H2:DIM_H], tnh_t[:, H2:DIM_H], OP.mult
                        )

                        # second cosine gate
                        dot2 = sm2_pool.tile([P, 1], fp32, tag="dot2")
                        dmp = dv_pool.tile(
                            [P, DIM_H], fp32, tag="dmp", name=f"dmp{t}"
                        )
                        nc.vector.scalar_tensor_tensor(
                            out=dmp,
                            in0=hxn_t,
                            scalar=1.0,
                            in1=iact[t],
                            op0=OP.mult,
                            op1=OP.mult,
                            accum_out=dot2,
                        )
                        sq1 = sm2_pool.tile([P, 1], fp32, tag="sq1")
                        dmpa = ps_ad.tile(
                            [P, DIM_H], fp32, tag="dmpa", name=f"dmpa{t}"
                        )
                        nc.scalar.activation(
                            dmpa, hxn_t, AF.Square, accum_out=sq1
                        )
                        sq2 = sq2s[t]
                        ma = sm2_pool.tile([P, 1], fp32, tag="ma")
                        nc.vector.tensor_scalar_max(ma, sq1, COS_EPS2)
                        mb = sm2_pool.tile([P, 1], fp32, tag="mb")
                        nc.vector.tensor_scalar_max(mb, sq2, COS_EPS2)
                        dn2 = sm2_pool.tile([P, 1], fp32, tag="dn2")
                        nc.vector.tensor_tensor(dn2, ma, mb, OP.mult)
                        rr2 = rsqrt_dve(sm2_pool, dn2, 1, "rs3")
                        arg2 = sm2_pool.tile([P, 1], fp32, tag="arg2")
                        nc.vector.tensor_scalar(
                            arg2, dot2, rr2, 0.5, OP.mult, OP.mult
                        )
                        co_t = sm2_pool.tile([P, 1], fp32, tag="co")
                        nc.scalar.activation(
                            co_t, arg2, AF.Sigmoid, bias=halfc
                        )
                        # hx_mod = hxn*co + hxn in one DVE pass
                        nc.vector.scalar_tensor_tensor(
                            out=hxn_t,
                            in0=hxn_t,
                            scalar=co_t,
                            in1=hxn_t,
                            op0=OP.mult,
                            op1=OP.add,
                        )
                        nc.scalar.dma_start(
                            out=hxo[t * P : (t + 1) * P, :], in_=hxn_t
                        )
    if split_waits:
        _split_excess_waits(nc)
    return nc


def _split_excess_waits(nc):
    """Walrus ISA structs have limited sync-wait slots (Matmult/LDW: 1,
    DMA: 2, several DVE/ACT structs: 1-2). The Tile scheduler can emit more.
    Move excess waits onto standalone EventSemaphore instructions injected
    just before the offender on the same engine."""
    import concourse.mybir as mybir

    caps = {}
    skip = {"EventSemaphore", "RegisterMove", "UnconditionalBranch"}
    n_split = 0
    for fn in nc.m.functions:
        for blk in fn.blocks:
            out = []
            changed = False
            for ins in blk.instructions:
                si = ins.sync_info
                opname = type(ins).__name__.replace("Inst", "", 1)
                if (
                    si is not None
                    and si.on_wait
                    and opname not in skip
                    and len(si.on_wait) > caps.get(opname, 1)
                ):
                    cap = caps.get(opname, 1)
                    waits = list(si.on_wait)
                    excess, keep = waits[:-cap], waits[-cap:]
                    for k, w in enumerate(excess):
                        ev = mybir.InstEventSemaphore(
                            name=f"{ins.name}-wsp{k}",
                            ins=[],
                            outs=[],
                            sync_info=mybir.SyncInfo(on_wait=[w], on_update=[]),
                        )
                        ev.engine = ins.engine
                        out.append(ev)
                        n_split += 1
                    ins.sync_info = mybir.SyncInfo(
                        on_wait=keep, on_update=list(si.on_update)
                    )
                    changed = True
                out.append(ins)
            if changed:
                blk.instructions = out
    return n_split


def _get_nc():
    if "nc" not in _cache:
        _cache["nc"] = build_nc()
    return _cache["nc"]


def kernel(x, hx, cx, W, b, Wm, bm, gammas, betas):
    from concourse.bass_utils import run_bass_kernel_spmd

    nc = _get_nc()
    x = np.ascontiguousarray(np.asarray(x, np.float32))
    hx = np.ascontiguousarray(np.asarray(hx, np.float32))
    cx = np.ascontiguousarray(np.asarray(cx, np.float32))
    shared = {
        "W": np.ascontiguousarray(np.asarray(W, np.float32)),
        "b": np.ascontiguousarray(np.asarray(b, np.float32)),
        "Wm": np.ascontiguousarray(np.asarray(Wm, np.float32)),
        "bm": np.ascontiguousarray(np.asarray(bm, np.float32)),
        "gammas": np.ascontiguousarray(np.asarray(gammas, np.float32)),
        "betas": np.ascontiguousarray(np.asarray(betas, np.float32)),
    }
    in_maps = []
    for i in range(NCORES):
        sl = slice(i * BL, (i + 1) * BL)
        in_maps.append({"x": x[sl], "hx": hx[sl], "cx": cx[sl], **shared})
    res = run_bass_kernel_spmd(nc, in_maps, list(range(NCORES)))
    hx_mod = np.concatenate([r["hx_out"] for r in res.results], axis=0)
    cx_new = np.concatenate([r["cx_out"] for r in res.results], axis=0)
    return (hx_mod, cx_new)


# revision 17
# speedup vs baseline: 2.8718x; 1.1346x over previous
"""Trainium2 Bass kernel for the cosine-gated LSTM cell (CGLSTMCellv1).

Full inputs in, full outputs out. Internally: data-parallel shard of the
batch dim across 8 NeuronCores, weights replicated, no cross-core comms.

Math per core (rows = local batch slice):
  mapped = x @ Wm + bm
  attn   = sigmoid(cos_sim(mapped, hx));  s = 1 + attn
  gates  = concat(s*x, hx) @ W + b  = s*(x@Wx) + hx@Wh + b  (s folded into xT)
  i,f,g,o = LN-gates -> sigmoid/tanh
  cx_new = f*cx + i*g ; hx_new = o*tanh(cx_new)
  hx_mod = hx_new * (1 + sigmoid((cos_sim(hx_new,cx_new)+1)/2))

Perf notes vs the fp32 baseline:
  - All GEMM matmuls run as float32r (1 cycle/row when N>=256, vs 4 for
    fp32) via AP.bitcast; PSUM accumulation stays fp32.
  - No ACT sqrt anywhere: rsqrt is a Quake-style bit hack + Newton steps
    on the Vector engine, so the ScalarE activation table stays on the
    sigmoid_and_others set (sigmoid/tanh/square/copy) the whole kernel —
    the baseline burned ~100us in ACT_TABLE_LOADs alternating sqrt<->
    sigmoid.
  - LayerNorm tail consumes PSUM directly: bn_stats on the psum chunks,
    final chunk never copied; apply is two fused scalar_tensor_tensor
    passes  u=(z-mu)*gamma ; w=u*rstd+beta  on DVE, activation on ACT.
  - GpSimd (no PSUM port) only does SBUF-side elementwise (gate combine).

Walrus codegen limits sync waits per instruction (Matmult: 1, DMA: 2), so
PSUM consumers are kept few, and dummy "absorber" transposes pre-observe
DMA semaphores; _split_excess_waits catches the rest.
"""

import numpy as np

B_FULL, DIM_I, DIM_H = 8192, 1024, 1024
NCORES = 8
BL = B_FULL // NCORES  # 1024 rows per core
P = 128
H4 = 4 * DIM_H
NKB1 = DIM_I // P            # 8  k-blocks for mm1
NKB2 = (DIM_I + DIM_H) // P  # 16 k-blocks for mm2
CHUNK = 512                  # W column chunk
NCH_G = DIM_H // CHUNK       # 4 chunks per gate
LN_EPS = 1e-5
COS_EPS2 = 1e-12
QMAGIC = 0x5F3759DF

_cache = {}


def build_nc(nbt=BL // P, split_waits=True):
    """Build the single-core Bass module; nbt = number of 128-row batch tiles."""
    from contextlib import ExitStack

    import concourse.bass as bass
    import concourse.mybir as mybir
    import concourse.tile as tile
    import concourse.tile_rust as tile_rust
    from concourse.masks import make_identity

    fp32 = mybir.dt.float32
    fp32r = mybir.dt.float32r
    bf16 = mybir.dt.bfloat16
    i32 = mybir.dt.int32
    AF = mybir.ActivationFunctionType
    OP = mybir.AluOpType
    bl = nbt * P

    def R(ap):
        return ap.bitcast(fp32r)

    nc = bass.Bass()
    xd = nc.dram_tensor("x", [bl, DIM_I], fp32, kind="ExternalInput")
    hxd = nc.dram_tensor("hx", [bl, DIM_H], fp32, kind="ExternalInput")
    cxd = nc.dram_tensor("cx", [bl, DIM_H], fp32, kind="ExternalInput")
    Wd = nc.dram_tensor("W", [DIM_I + DIM_H, H4], fp32r, kind="ExternalInput")
    bd = nc.dram_tensor("b", [H4], fp32r, kind="ExternalInput")
    Wmd = nc.dram_tensor("Wm", [DIM_I, DIM_H], fp32r, kind="ExternalInput")
    bmd = nc.dram_tensor("bm", [DIM_H], fp32r, kind="ExternalInput")
    gd = nc.dram_tensor("gammas", [4, DIM_H], fp32, kind="ExternalInput")
    btd = nc.dram_tensor("betas", [4, DIM_H], fp32, kind="ExternalInput")
    hxo = nc.dram_tensor("hx_out", [bl, DIM_H], fp32, kind="ExternalOutput")
    cxo = nc.dram_tensor("cx_out", [bl, DIM_H], fp32, kind="ExternalOutput")

    def bcast_row(src_ap):
        # view an [N]-shaped AP as [P, N] with 0-step partition broadcast
        return bass.AP(
            tensor=src_ap.tensor, offset=src_ap.offset, ap=[[0, P]] + list(src_ap.ap)
        )

    def raw(inst):
        return getattr(inst, "ins", inst)

    with tile.TileContext(nc) as tc, ExitStack() as ctx:
        singles = ctx.enter_context(tc.tile_pool(name="singles", bufs=1))

        ident = singles.tile([P, P], fp32)
        make_identity(nc, ident)
        ident_r = singles.tile([P, P], fp32r)
        nc.scalar.copy(ident_r, ident)
        ones128 = singles.tile([P, P], fp32)
        nc.vector.memset(ones128, 1.0)
        zrow = singles.tile([P, P], fp32)
        nc.vector.memset(zrow, 0.0)
        halfc = singles.tile([P, 1], fp32)
        nc.vector.memset(halfc, 0.5)
        one_i = singles.tile([P, 1], i32)
        nc.vector.memset(one_i, 1)
        magic_i = singles.tile([P, 1], i32)
        i_msl = nc.vector.memset(magic_i, QMAGIC)

        # transposed activations, persistent across both phases
        xsT_all = singles.tile([P, nbt, NKB1, P], bf16)
        hxT_all = singles.tile([P, nbt, NKB1, P], bf16)

        Wv = Wd[:].rearrange("(kb p) n -> p kb n", p=P)
        Wmv = Wmd[:].rearrange("(kb p) n -> p kb n", p=P)

        def colbrd(src, n):
            # view a [P,1] tile as [P,n] with 0-stride free broadcast
            ap = src[:, 0:1]
            return bass.AP(
                tensor=ap.tensor, offset=ap.offset, ap=[list(ap.ap[0]), [0, n]]
            )

        def rsqrt_dve(pool, v_ap, iters, tag):
            """1/sqrt(v) on DVE: Quake bit hack + `iters` Newton steps.
            v_ap: [P,n] fp32 AP. Returns a [P,n] fp32 tile."""
            n = v_ap.free_size()
            vi = v_ap.bitcast(i32)
            y = pool.tile([P, n], fp32, tag=f"{tag}_y")
            yi = y.bitcast(i32)
            t0 = pool.tile([P, n], i32, tag=f"{tag}_t0")
            nc.vector.tensor_tensor(t0, vi, colbrd(one_i, n), OP.logical_shift_right)
            nc.vector.tensor_tensor(yi, colbrd(magic_i, n), t0, OP.subtract)
            for _ in range(iters):
                a = pool.tile([P, n], fp32, tag=f"{tag}_a")
                nc.vector.tensor_tensor(a, v_ap, y, OP.mult)
                nc.vector.tensor_tensor(a, a, y, OP.mult)
                nc.vector.tensor_scalar(a, a, -0.5, 1.5, OP.mult, OP.add)
                nc.vector.tensor_tensor(y, y, a, OP.mult)
            return y

        def absorber(ps_tile):
            def absorb(dep_inst=None):
                """Dummy PE transpose pre-observing one semaphore so real
                matmuls never need more than one sync wait (S3_LW limit)."""
                di = nc.tensor.transpose(ps_tile, ident, ident)
                if dep_inst is not None:
                    tile_rust.add_dep_helper(
                        raw(di), raw(dep_inst), reason="absorb sem for PE"
                    )
                return di

            return absorb

        # ---------------- phase 1 ----------------
        with ExitStack() as p1:
            wm_pool = p1.enter_context(tc.tile_pool(name="wm", bufs=1))
            io_pool = p1.enter_context(tc.tile_pool(name="io1", bufs=2))
            sm_pool = p1.enter_context(tc.tile_pool(name="smalls1", bufs=4))
            dump_pool = p1.enter_context(tc.tile_pool(name="dump1", bufs=3))
            ps_tr = p1.enter_context(tc.tile_pool(name="pstr", bufs=2, space="PSUM"))
            ps_sm = p1.enter_context(tc.tile_pool(name="pssm", bufs=1, space="PSUM"))
            ps_m1 = p1.enter_context(tc.tile_pool(name="psm1", bufs=2, space="PSUM"))

            bm_rep = wm_pool.tile([P, DIM_H], fp32r)
            i_bm = nc.sync.dma_start(out=bm_rep, in_=bcast_row(bmd[:]))
            wm_sb = wm_pool.tile([P, NKB1, DIM_H], fp32r)
            i_wm = nc.sync.dma_start(out=wm_sb, in_=Wmv)

            x_all = wm_pool.tile([P, nbt, DIM_I], fp32)
            hx_all = wm_pool.tile([P, nbt, DIM_H], fp32)
            xload, hxload = [], []
            for t in range(nbt):
                xload.append(
                    nc.sync.dma_start(
                        out=x_all[:, t], in_=xd[t * P : (t + 1) * P, :]
                    )
                )
                hxload.append(
                    nc.sync.dma_start(
                        out=hx_all[:, t], in_=hxd[t * P : (t + 1) * P, :]
                    )
                )

            dmy = ps_sm.tile([P, P], fp32, tag="dmy", bufs=1, name="dmy")
            absorb = absorber(dmy)
            absorb()  # ident (gpsimd sem)
            absorb(i_msl)  # vector memsets
            absorb(i_bm)  # bm_rep dma queue
            absorb(i_wm)  # wm dma queue

            cp_insts = []
            for t in range(nbt):
                x_t = x_all[:, t]
                hx_t = hx_all[:, t]

                absorb(xload[t])
                xT_t = io_pool.tile([P, NKB1, P], fp32r, tag="xT_t")
                for h in range(2):
                    pt = ps_tr.tile([P, 512], fp32, tag="tr", name=f"ptx{t}_{h}")
                    for j in range(4):
                        jj = h * 4 + j
                        nc.tensor.transpose(
                            pt[:, j * P : (j + 1) * P],
                            x_t[:, jj * P : (jj + 1) * P],
                            ident,
                        )
                    nc.scalar.copy(xT_t[:, h * 4 : (h + 1) * 4, :], pt)
                absorb(hxload[t])
                for h in range(2):
                    pt = ps_tr.tile([P, 512], fp32, tag="tr", name=f"pth{t}_{h}")
                    for j in range(4):
                        jj = h * 4 + j
                        nc.tensor.transpose(
                            pt[:, j * P : (j + 1) * P],
                            hx_t[:, jj * P : (jj + 1) * P],
                            ident,
                        )
                    nc.scalar.copy(hxT_all[:, t, h * 4 : (h + 1) * 4, :], pt)

                # mm1: mapped = bm + x @ Wm   (psum [P, 1024], two N=512 groups)
                if t >= 2:
                    for ci in cp_insts[t - 2]:
                        absorb(ci)  # pm slot release (DVE dot + ACT square)
                pm = ps_m1.tile([P, DIM_H], fp32, tag="pm1", name=f"pm{t}")
                for nh in range(2):
                    cs = slice(nh * 512, (nh + 1) * 512)
                    nc.tensor.matmul(
                        pm[:, cs], ident_r, bm_rep[:, cs], start=True, stop=False
                    )
                    for kb in range(NKB1):
                        nc.tensor.matmul(
                            pm[:, cs],
                            xT_t[:, kb, :],
                            wm_sb[:, kb, cs],
                            start=False,
                            stop=(kb == NKB1 - 1),
                        )

                # cosine attention gate; DVE dot + ACT square read the psum
                dot_t = sm_pool.tile([P, 1], fp32, tag="dot")
                dmp0 = dump_pool.tile([P, DIM_H], fp32, tag="dump")
                i_dot = nc.vector.scalar_tensor_tensor(
                    out=dmp0,
                    in0=pm,
                    scalar=1.0,
                    in1=hx_t,
                    op0=OP.mult,
                    op1=OP.mult,
                    accum_out=dot_t,
                )
                sqm_t = sm_pool.tile([P, 1], fp32, tag="sqm")
                dmp1 = dump_pool.tile([P, DIM_H], fp32, tag="dump")
                i_sqm = nc.scalar.activation(dmp1, pm, AF.Square, accum_out=sqm_t)
                cp_insts.append((i_dot, i_sqm))
                sqh_t = sm_pool.tile([P, 1], fp32, tag="sqh")
                dmp2 = dump_pool.tile([P, DIM_H], fp32, tag="dump")
                nc.scalar.activation(dmp2, hx_t, AF.Square, accum_out=sqh_t)

                m1_t = sm_pool.tile([P, 1], fp32, tag="m1")
                nc.vector.tensor_scalar_max(m1_t, sqm_t, COS_EPS2)
                m2_t = sm_pool.tile([P, 1], fp32, tag="m2")
                nc.vector.tensor_scalar_max(m2_t, sqh_t, COS_EPS2)
                den_t = sm_pool.tile([P, 1], fp32, tag="den")
                nc.vector.tensor_tensor(den_t, m1_t, m2_t, OP.mult)
                rinv_t = rsqrt_dve(sm_pool, den_t, 1, "rs1")
                cos_t = sm_pool.tile([P, 1], fp32, tag="cos")
                nc.vector.tensor_scalar_mul(cos_t, dot_t, rinv_t)
                attn_t = sm_pool.tile([P, 1], fp32, tag="attn")
                nc.scalar.activation(attn_t, cos_t, AF.Sigmoid)

                # transpose attn -> row 0 of zrow, replicate via ones-matmul
                psT = ps_sm.tile([1, P], fp32, tag="paux", name=f"psT{t}")
                nc.tensor.transpose(psT, attn_t, ident)
                nc.scalar.copy(zrow[0:1, :], psT)
                psr = ps_sm.tile([P, P], fp32, tag="paux", name=f"psr{t}")
                nc.tensor.matmul(psr, ones128, zrow, start=True, stop=True)
                srep_t = sm_pool.tile([P, P], fp32, tag="srep")
                nc.scalar.copy(srep_t, psr)

                srep_brd = bass.AP(
                    tensor=srep_t.tensor,
                    offset=srep_t.offset,
                    ap=[list(srep_t.ap[0]), [0, NKB1], list(srep_t.ap[1])],
                )
                # xsT = (1 + attn) * xT in one DVE pass
                nc.vector.scalar_tensor_tensor(
                    out=xsT_all[:, t],
                    in0=srep_brd,
                    scalar=1.0,
                    in1=xT_t,
                    op0=OP.add,
                    op1=OP.mult,
                )


        # ---------------- phase 2 ----------------
        with ExitStack() as p2:
            w32_pool = p2.enter_context(tc.tile_pool(name="w32", bufs=2))
            wb_pool = p2.enter_context(tc.tile_pool(name="wchb", bufs=2))
            bsl_pool = p2.enter_context(tc.tile_pool(name="bsl", bufs=2))
            gb_pool = p2.enter_context(tc.tile_pool(name="gb", bufs=1))
            iact_pool = p2.enter_context(tc.tile_pool(name="iact", bufs=nbt))
            zst_pool = p2.enter_context(tc.tile_pool(name="zst", bufs=nbt))
            u_pool = p2.enter_context(tc.tile_pool(name="u", bufs=2))
            tnh_pool = p2.enter_context(tc.tile_pool(name="tnh", bufs=2))
            st_pool = p2.enter_context(tc.tile_pool(name="stats", bufs=nbt + 2))
            v_pool = p2.enter_context(tc.tile_pool(name="vall", bufs=2))
            cx_pool = p2.enter_context(tc.tile_pool(name="cxin", bufs=3))
            dv_pool = p2.enter_context(tc.tile_pool(name="dvdump", bufs=2))
            sm2_pool = p2.enter_context(tc.tile_pool(name="smalls2", bufs=2))
            sq2_pool = p2.enter_context(tc.tile_pool(name="sq2p", bufs=nbt))
            ps_g = p2.enter_context(tc.tile_pool(name="psg", bufs=5, space="PSUM"))
            ps_ad = p2.enter_context(
                tc.tile_pool(name="psact", bufs=1, space="PSUM")
            )

            dmy2 = ps_g.tile([P, P], fp32, tag="dmy2", bufs=1, name="dmy2")
            absorb2 = absorber(dmy2)

            iact = [
                iact_pool.tile([P, DIM_H], fp32, tag="iact", name=f"iact{t}")
                for t in range(nbt)
            ]
            zst = [
                zst_pool.tile([P, DIM_H], bf16, tag="zst", name=f"zst{t}")
                for t in range(nbt)
            ]
            stats = [
                st_pool.tile([P, NCH_G, 6], fp32, tag="st", name=f"st{t}")
                for t in range(nbt)
            ]

            sq2s = []
            # gate order: i first (stored), then g (i*g), f (cx_new), o (outputs)
            for gi, func, role in (
                (0, AF.Sigmoid, "i"),
                (2, AF.Tanh, "g"),
                (1, AF.Sigmoid, "f"),
                (3, AF.Sigmoid, "o"),
            ):
                vall = v_pool.tile([P, nbt, 2], fp32, tag="vall", name=f"vall{gi}")
                grep_t = gb_pool.tile([P, DIM_H], fp32, tag="grep", name=f"grep{gi}")
                brep_t = gb_pool.tile([P, DIM_H], fp32, tag="brep", name=f"brep{gi}")

                for c in range(NCH_G):
                    col0 = gi * DIM_H + c * CHUNK
                    # stream W as fp32r halves, convert to bf16 (DVE/ACT split)
                    wchb = wb_pool.tile(
                        [P, NKB2, CHUNK], bf16, tag="wchb", name=f"wchb{gi}_{c}"
                    )
                    for h in range(2):
                        w32 = w32_pool.tile(
                            [P, NKB2, CHUNK // 2],
                            fp32r,
                            tag="w32",
                            name=f"w32_{gi}_{c}_{h}",
                        )
                        hc = col0 + h * (CHUNK // 2)
                        nc.sync.dma_start(
                            out=w32, in_=Wv[:, :, hc : hc + CHUNK // 2]
                        )
                        dst = wchb[:, :, h * (CHUNK // 2) : (h + 1) * (CHUNK // 2)]
                        if h == 0:
                            nc.vector.tensor_copy(dst, w32)
                        else:
                            nc.scalar.copy(dst, w32)
                    if c == 0:
                        # gamma/beta after the gate's first W DMAs: their slot
                        # wait can't block the W stream at gate boundaries
                        nc.sync.dma_start(out=grep_t, in_=bcast_row(gd[gi, :]))
                        nc.sync.dma_start(out=brep_t, in_=bcast_row(btd[gi, :]))
                    bsl = bsl_pool.tile(
                        [P, CHUNK], fp32r, tag="bsl", name=f"bsl{gi}_{c}"
                    )
                    i_bsl = nc.sync.dma_start(
                        out=bsl, in_=bcast_row(bd[col0 : col0 + CHUNK])
                    )
                    absorb2(i_bsl)

                    for t in range(nbt):
                        ps = ps_g.tile(
                            [P, CHUNK], fp32, tag="pg", name=f"pg{gi}_{c}_{t}"
                        )
                        nc.tensor.matmul(ps, ident_r, bsl, start=True, stop=False)
                        for kb in range(NKB2):
                            lhsT = (
                                xsT_all[:, t, kb, :]
                                if kb < NKB1
                                else hxT_all[:, t, kb - NKB1, :]
                            )
                            nc.tensor.matmul(
                                ps,
                                lhsT,
                                wchb[:, kb, :],
                                start=False,
                                stop=(kb == NKB2 - 1),
                            )
                        nc.vector.bn_stats(stats[t][:, c, :], ps)
                        nc.scalar.copy(zst[t][:, c * CHUNK : (c + 1) * CHUNK], ps)
                        if c == NCH_G - 1:
                            nc.vector.bn_aggr(vall[:, t, :], stats[t])

                # batched LN scalars for all tiles of this gate
                veps_t = sm2_pool.tile([P, nbt], fp32, tag="veps")
                nc.vector.tensor_scalar_add(veps_t, vall[:, :, 1:2], LN_EPS)
                rstd_all = rsqrt_dve(sm2_pool, veps_t, 2, "rs2")
                nmu_all = sm2_pool.tile([P, nbt], fp32, tag="nmu")
                nc.vector.tensor_scalar_mul(nmu_all, vall[:, :, 0:1], -1.0)

                for t in range(nbt):
                    # u = (z - mu) * gamma ; w = u * rstd + beta (in place)
                    u_t = u_pool.tile([P, DIM_H], fp32, tag="u", name=f"u{gi}_{t}")
                    nc.vector.scalar_tensor_tensor(
                        out=u_t,
                        in0=zst[t],
                        scalar=nmu_all[:, t : t + 1],
                        in1=grep_t,
                        op0=OP.add,
                        op1=OP.mult,
                    )
                    nc.vector.scalar_tensor_tensor(
                        out=u_t,
                        in0=u_t,
                        scalar=rstd_all[:, t : t + 1],
                        in1=brep_t,
                        op0=OP.mult,
                        op1=OP.add,
                    )
                    if role == "i":
                        nc.scalar.activation(iact[t], u_t, func)
                    else:
                        nc.scalar.activation(u_t, u_t, func)
                        ga = u_t

                    if role == "g":
                        nc.gpsimd.tensor_tensor(iact[t], iact[t], ga, OP.mult)
                    elif role == "f":
                        cx_t = cx_pool.tile(
                            [P, DIM_H], fp32, tag="cx", name=f"cx{t}"
                        )
                        nc.sync.dma_start(
                            out=cx_t, in_=cxd[t * P : (t + 1) * P, :]
                        )
                        nc.gpsimd.tensor_tensor(cx_t, ga, cx_t, OP.mult)
                        nc.gpsimd.tensor_tensor(iact[t], iact[t], cx_t, OP.add)
                        nc.scalar.dma_start(
                            out=cxo[t * P : (t + 1) * P, :], in_=iact[t]
                        )
                        sq2 = sq2_pool.tile([P, 1], fp32, tag="sq2", name=f"sq2_{t}")
                        sq2s.append(sq2)
                        dmpb = ps_ad.tile(
                            [P, DIM_H], fp32, tag="dmpa", name=f"dmpb{t}"
                        )
                        nc.scalar.activation(
                            dmpb, iact[t], AF.Square, accum_out=sq2
                        )
                    elif role == "o":
                        tnh_t = tnh_pool.tile(
                            [P, DIM_H], fp32, tag="tnh", name=f"tnh{t}"
                        )
                        nc.scalar.activation(tnh_t, iact[t], AF.Tanh)
                        # hx_new in place of tanh(cx_new); split halves
                        hxn_t = tnh_t
                        H2 = DIM_H // 2
                        nc.gpsimd.tensor_tensor(
                            hxn_t[:, 0:H2], ga[:, 0:H2], tnh_t[:, 0:H2], OP.mult
                        )
                        nc.vector.tensor_tensor(
                            hxn_t[:, H2:DIM_H], ga[:, H2:DIM_H], tnh_t[:, H2:DIM_H], OP.mult
                        )

                        # second cosine gate
                        dot2 = sm2_pool.tile([P, 1], fp32, tag="dot2")
                        dmp = dv_pool.tile(
                            [P, DIM_H], fp32, tag="dmp", name=f"dmp{t}"
                        )
                        nc.vector.scalar_tensor_tensor(
                            out=dmp,
                            in0=hxn_t,
                            scalar=1.0,
                            in1=iact[t],
                            op0=OP.mult,
                            op1=OP.mult,
                            accum_out=dot2,
                        )
                        sq1 = sm2_pool.tile([P, 1], fp32, tag="sq1")
                        dmpa = ps_ad.tile(
                            [P, DIM_H], fp32, tag="dmpa", name=f"dmpa{t}"
                        )
                        nc.scalar.activation(
                            dmpa, hxn_t, AF.Square, accum_out=sq1
                        )
                        sq2 = sq2s[t]
                        ma = sm2_pool.tile([P, 1], fp32, tag="ma")
                        nc.vector.tensor_scalar_max(ma, sq1, COS_EPS2)
                        mb = sm2_pool.tile([P, 1], fp32, tag="mb")
                        nc.vector.tensor_scalar_max(mb, sq2, COS_EPS2)
                        dn2 = sm2_pool.tile([P, 1], fp32, tag="dn2")
                        nc.vector.tensor_tensor(dn2, ma, mb, OP.mult)
                        rr2 = rsqrt_dve(sm2_pool, dn2, 1, "rs3")
                        arg2 = sm2_pool.tile([P, 1], fp32, tag="arg2")
                        nc.vector.tensor_scalar(
                            arg2, dot2, rr2, 0.5, OP.mult, OP.mult
                        )
                        co_t = sm2_pool.tile([P, 1], fp32, tag="co")
                        nc.scalar.activation(
                            co_t, arg2, AF.Sigmoid, bias=halfc
                        )
                        # hx_mod = hxn*co + hxn in one DVE pass
                        nc.vector.scalar_tensor_tensor(
                            out=hxn_t,
                            in0=hxn_t,
                            scalar=co_t,
                            in1=hxn_t,
                            op0=OP.mult,
                            op1=OP.add,
                        )
                        nc.scalar.dma_start(
                            out=hxo[t * P : (t + 1) * P, :], in_=hxn_t
                        )
    if split_waits:
        _split_excess_waits(nc)
    return nc


def _split_excess_waits(nc):
    """Walrus ISA structs have limited sync-wait slots (Matmult/LDW: 1,
    DMA: 2, several DVE/ACT structs: 1-2). The Tile scheduler can emit more.
    Move excess waits onto standalone EventSemaphore instructions injected
    just before the offender on the same engine."""
    import concourse.mybir as mybir

    caps = {}
    skip = {"EventSemaphore", "RegisterMove", "UnconditionalBranch"}
    n_split = 0
    for fn in nc.m.functions:
        for blk in fn.blocks:
            out = []
            changed = False
            for ins in blk.instructions:
                si = ins.sync_info
                opname = type(ins).__name__.replace("Inst", "", 1)
                if (
                    si is not None
                    and si.on_wait
                    and opname not in skip
                    and len(si.on_wait) > caps.get(opname, 1)
                ):
                    cap = caps.get(opname, 1)
                    waits = list(si.on_wait)
                    excess, keep = waits[:-cap], waits[-cap:]
                    for k, w in enumerate(excess):
                        ev = mybir.InstEventSemaphore(
                            name=f"{ins.name}-wsp{k}",
                            ins=[],
                            outs=[],
                            sync_info=mybir.SyncInfo(on_wait=[w], on_update=[]),
                        )
                        ev.engine = ins.engine
                        out.append(ev)
                        n_split += 1
                    ins.sync_info = mybir.SyncInfo(
                        on_wait=keep, on_update=list(si.on_update)
                    )
                    changed = True
                out.append(ins)
            if changed:
                blk.instructions = out
    return n_split


def _get_nc():
    if "nc" not in _cache:
        _cache["nc"] = build_nc()
    return _cache["nc"]


def kernel(x, hx, cx, W, b, Wm, bm, gammas, betas):
    from concourse.bass_utils import run_bass_kernel_spmd

    nc = _get_nc()
    x = np.ascontiguousarray(np.asarray(x, np.float32))
    hx = np.ascontiguousarray(np.asarray(hx, np.float32))
    cx = np.ascontiguousarray(np.asarray(cx, np.float32))
    shared = {
        "W": np.ascontiguousarray(np.asarray(W, np.float32)),
        "b": np.ascontiguousarray(np.asarray(b, np.float32)),
        "Wm": np.ascontiguousarray(np.asarray(Wm, np.float32)),
        "bm": np.ascontiguousarray(np.asarray(bm, np.float32)),
        "gammas": np.ascontiguousarray(np.asarray(gammas, np.float32)),
        "betas": np.ascontiguousarray(np.asarray(betas, np.float32)),
    }
    in_maps = []
    for i in range(NCORES):
        sl = slice(i * BL, (i + 1) * BL)
        in_maps.append({"x": x[sl], "hx": hx[sl], "cx": cx[sl], **shared})
    res = run_bass_kernel_spmd(nc, in_maps, list(range(NCORES)))
    hx_mod = np.concatenate([r["hx_out"] for r in res.results], axis=0)
    cx_new = np.concatenate([r["cx_out"] for r in res.results], axis=0)
    return (hx_mod, cx_new)
